# revision 16
# baseline (speedup 1.0000x reference)
"""MultiHeadSectionAttentionImputer on 8 TRN2 NeuronCores (Bass/Tile).

Sharding: the N=6144 existing sections are split across the 8 cores
(768 each). Each core:
  - projects its exist-shard to K,V  (K_loc = X_e @ Wk, V_loc = X_e @ Wv + ones col)
  - projects the full missing set to Q (duplicated across cores; Wq,bq
    pre-scaled by 1/sqrt(d_k) on host)
  - computes scoresT[n,m] per head with a fused 128-deep contraction:
      d' = [q-dims(64) | cooc-bias-dims(64)]  ->  q.k/sqrt(dk) + mb.eb
  - exp() without max subtraction (scores are bounded ~<60; fp32 range ok)
  - partial out^T = attn @ [V | 1]  ->  numerator (64 cols) + denominator
Host combines partial numerators/denominators across cores (softmax over
the full key set), adds bv, and scatters into a copy of ehr_embeddings.

All matmul inputs are float32r (tf32-like, full-rate on PE) except the
attention-weight matmul which uses bf16 (exp output cast, attn in [0, e^60]).
"""

import os
import sys
import numpy as np
from contextlib import ExitStack

sys.path.insert(0, "/opt/trn_rl_repo")

# problem constants (hardcoded; kernel.py must be self-contained)
H = 12          # heads
DK = 64         # head dim
E = 768         # embed dim
TOTAL = H * DK  # 768
M = 2048        # missing sections
N = 6144        # existing sections
S = 8192        # total sections
CORES = 8
NLOC = N // CORES        # 768 keys per core
EC = E // 128            # 6 contraction chunks
NI = NLOC // 128         # 6 key chunks per core
MI = M // 128            # 16 query chunks
PAIRS = H // 2           # 6 head pairs

_CACHE = {}
LAST_EXEC_NS = None
LAST_TRACE_DIR = None


def _build():
    import concourse.bass as bass
    import concourse.tile as tile
    from concourse import bacc, mybir

    F32 = mybir.dt.float32
    FP16 = mybir.dt.float16
    BF16 = mybir.dt.bfloat16
    Exp = mybir.ActivationFunctionType.Exp

    nc = bacc.Bacc("TRN2", target_bir_lowering=False, debug=False)

    # ---- I/O ----
    xt_m = nc.dram_tensor("xt_m", [128, EC, M], FP16, kind="ExternalInput").ap()
    mbt = nc.dram_tensor("mbt", [H * DK, M], FP16, kind="ExternalInput").ap()
    xt_e = nc.dram_tensor("xt_e", [128, EC, NLOC], FP16, kind="ExternalInput").ap()
    ebt = nc.dram_tensor("ebt", [H * DK, NLOC], FP16, kind="ExternalInput").ap()
    wq = nc.dram_tensor("wq", [128, EC, TOTAL], FP16, kind="ExternalInput").ap()
    wk = nc.dram_tensor("wk", [128, EC, TOTAL], FP16, kind="ExternalInput").ap()
    wv = nc.dram_tensor("wv", [128, EC, TOTAL], FP16, kind="ExternalInput").ap()
    bq = nc.dram_tensor("bq", [128, PAIRS], F32, kind="ExternalInput").ap()
    out_p = nc.dram_tensor("out_p", [H, M, DK + 1], F32, kind="ExternalOutput").ap()

    with tile.TileContext(nc) as tc, ExitStack() as ctx:
        persist = ctx.enter_context(tc.tile_pool(name="persist", bufs=1))
        qpt_pool = ctx.enter_context(tc.tile_pool(name="qpt", bufs=4))
        attn_pool = ctx.enter_context(tc.tile_pool(name="attn", bufs=18))
        osb_pool = ctx.enter_context(tc.tile_pool(name="osb", bufs=12))
        proj_ps = ctx.enter_context(tc.tile_pool(name="proj_ps", bufs=1, space="PSUM"))
        sc_ps = ctx.enter_context(tc.tile_pool(name="sc_ps", bufs=2, space="PSUM"))
        av_ps = ctx.enter_context(tc.tile_pool(name="av_ps", bufs=3, space="PSUM"))

        # input DMAs spread across queues, pieces ordered first-needed-first.
        # Only pair-0 column slices of wk/wq are on the critical path:
        #   sync:   wk[:,0:128] (kt0 lhsT), wk rest, xte col-rest
        #   scalar: bq, xte piece0 + xtm piece0 (kt0/qt0 rhs), xtm rest
        #   gpsimd: mbt h0/h1 + ebt p0 (q0/k0 bias rows), wq piece0, wv, wq rest
        kpt = [persist.tile([128, NLOC], FP16, tag=f"kpt{h}", name=f"kpt{h}") for h in range(H)]
        vsb = [persist.tile([128, H, DK + 1], BF16, tag=f"v{ni}", name=f"v{ni}") for ni in range(NI)]
        bq_sb = persist.tile([128, PAIRS], F32, tag="bq")
        nc.scalar.dma_start(bq_sb[:], bq)
        q0_0 = qpt_pool.tile([128, M], FP16, tag="qpt", name="qpt0")
        q1_0 = qpt_pool.tile([128, M], FP16, tag="qpt", name="qpt1")
        nc.gpsimd.dma_start(q0_0[64:128, :], mbt[0:DK, :])
        nc.gpsimd.dma_start(q1_0[0:64, :], mbt[DK:2 * DK, :])
        nc.gpsimd.dma_start(kpt[0][64:128, :], ebt[0:DK, :])
        nc.gpsimd.dma_start(kpt[1][0:64, :], ebt[DK:2 * DK, :])
        wk_big = persist.tile([128, EC, TOTAL], FP16, tag="wk")
        wq_big = persist.tile([128, EC, TOTAL], FP16, tag="wq")
        wv_big = persist.tile([128, EC, TOTAL], FP16, tag="wv")
        xte_big = persist.tile([128, EC, NLOC], FP16, tag="xte")
        xtm_big = persist.tile([128, EC, M], FP16, tag="xtm")
        # critical pieces first (kt0: wk cols 0:128 + xte cols 0:512;
        # qt0: wq cols 0:128 + xtm cols 0:512; then the remainders)
        nc.sync.dma_start(wk_big[:, :, 0:128], wk[:, :, 0:128])
        nc.scalar.dma_start(xte_big[:, :, 0:512], xt_e[:, :, 0:512])
        nc.scalar.dma_start(xtm_big[:, :, 0:512], xt_m[:, :, 0:512])
        nc.gpsimd.dma_start(wq_big[:, :, 0:128], wq[:, :, 0:128])
        nc.sync.dma_start(wk_big[:, :, 128:TOTAL], wk[:, :, 128:TOTAL])
        nc.scalar.dma_start(xtm_big[:, :, 512:M], xt_m[:, :, 512:M])
        nc.gpsimd.dma_start(wv_big[:], wv[:])
        nc.sync.dma_start(xte_big[:, :, 512:NLOC], xt_e[:, :, 512:NLOC])
        nc.gpsimd.dma_start(wq_big[:, :, 128:TOTAL], wq[:, :, 128:TOTAL])
        wk_sb = [wk_big[:, ec, :] for ec in range(EC)]
        wq_sb = [wq_big[:, ec, :] for ec in range(EC)]
        wv_sb = [wv_big[:, ec, :] for ec in range(EC)]
        xte_sb = [xte_big[:, ec, :] for ec in range(EC)]
        xtm_sb = [xtm_big[:, ec, :] for ec in range(EC)]

        def emit_scores_exp_half(h, qt, ni, half, at):
            """scoresT half-chunk [128 keys, 1024 queries] + exp into attnT.
            Two halves -> the 2-bank scores psum double-buffers, keeping
            ACT busy back-to-back instead of waiting a full 4-matmul round."""
            ps = sc_ps.tile([128, 1024], F32, tag="sc", name="sc_ps_t")
            mo = half * 1024
            for mj in range(2):
                nc.tensor.matmul(
                    ps[:, mj * 512:(mj + 1) * 512],
                    lhsT=kpt[h][:, ni * 128:(ni + 1) * 128],
                    rhs=qt[:, mo + mj * 512:mo + (mj + 1) * 512],
                    start=True, stop=True)
            nc.scalar.activation(at[:, mo:mo + 1024], ps[:], Exp)

        def emit_av(h, attns, g):
            """out chunks [128 queries, DK+1] for head h, mi in [2g, 2g+2)."""
            for mi in range(2 * g, 2 * g + 2):
                ps = av_ps.tile([128, DK + 1], F32, tag="av", name="av_ps_t")
                for ni in range(NI):
                    nc.tensor.matmul(
                        ps[:], lhsT=attns[ni][:, mi * 128:(mi + 1) * 128],
                        rhs=vsb[ni][:, h, :],
                        start=(ni == 0), stop=(ni == NI - 1))
                ot = osb_pool.tile([128, DK + 1], F32, tag="osb", name="osb_t")
                nc.vector.tensor_copy(ot[:], ps[:])
                nc.sync.dma_start(out_p[h, mi * 128:(mi + 1) * 128, :], ot[:])

        # ---- emission schedule ----
        # Unit-queue of (pe_cost_us, fn): drained between scores/exp
        # emissions under a per-sub-slot budget so the PE fills the exp
        # pipeline gaps without pushing the next scores matmuls far back
        # in its (in-order) stream.
        from collections import deque
        units = deque()
        qts = {0: q0_0, 1: q1_0}
        pair_ready = {0: 0}  # pair -> emitted kt+qt piece count (6 = ready)

        def qt_unit(p, mh):
            def f():
                pair_ready[p] = pair_ready.get(p, 0) + 1
                q0, q1 = qts.get(2 * p), qts.get(2 * p + 1)
                if q0 is None:
                    q0 = qpt_pool.tile([128, M], FP16, tag="qpt", name=f"qpt{2*p}")
                    q1 = qpt_pool.tile([128, M], FP16, tag="qpt", name=f"qpt{2*p+1}")
                    h0, h1 = 2 * p, 2 * p + 1
                    nc.sync.dma_start(q0[64:128, :], mbt[h0 * DK:(h0 + 1) * DK, :])
                    nc.sync.dma_start(q1[0:64, :], mbt[h1 * DK:(h1 + 1) * DK, :])
                    qts[2 * p], qts[2 * p + 1] = q0, q1
                emit_qt_half(p, mh, q0, q1)
            return (1.3, f)

        def emit_qt_half(p, mh, q0, q1):
            ps = proj_ps.tile([128, 512], F32, tag="proj", name="proj_qt")
            mo = mh * 512
            for ec in range(EC):
                nc.tensor.matmul(ps[:], lhsT=wq_sb[ec][:, p * 128:(p + 1) * 128],
                                 rhs=xtm_sb[ec][:, mo:mo + 512],
                                 start=(ec == 0), stop=(ec == EC - 1))
            nc.vector.tensor_scalar_add(
                q0[0:64, mo:mo + 512], ps[0:64, :], bq_sb[0:64, p:p + 1])
            nc.vector.tensor_scalar_add(
                q1[64:128, mo:mo + 512], ps[64:128, :], bq_sb[64:128, p:p + 1])

        def kt_unit(p, half):
            def f():
                pair_ready[p] = pair_ready.get(p, 0) + 1
                emit_kt_half(p, half)
            return (1.4, f)

        def emit_kt_half(p, half):
            h0, h1 = 2 * p, 2 * p + 1
            lo, hi = (0, 512) if half == 0 else (512, NLOC)
            ps = proj_ps.tile([128, 512], F32, tag="proj", name="proj_kt")
            for ec in range(EC):
                nc.tensor.matmul(ps[:, 0:hi - lo], lhsT=wk_sb[ec][:, p * 128:(p + 1) * 128],
                                 rhs=xte_sb[ec][:, lo:hi], start=(ec == 0), stop=(ec == EC - 1))
            nc.vector.tensor_copy(kpt[h0][0:64, lo:hi], ps[0:64, 0:hi - lo])
            nc.vector.tensor_copy(kpt[h1][64:128, lo:hi], ps[64:128, 0:hi - lo])
            if half == 0 and p > 0:
                nc.sync.dma_start(kpt[h0][64:128, :], ebt[h0 * DK:(h0 + 1) * DK, :])
                nc.sync.dma_start(kpt[h1][0:64, :], ebt[h1 * DK:(h1 + 1) * DK, :])

        def v_unit(ni, half):
            def f():
                lo, hi = (0, 512) if half == 0 else (512, TOTAL)
                ps = proj_ps.tile([128, 512], F32, tag="proj", name="proj_v")
                for ec in range(EC):
                    nc.tensor.matmul(ps[:, 0:hi - lo],
                                     lhsT=xte_sb[ec][:, ni * 128:(ni + 1) * 128],
                                     rhs=wv_sb[ec][:, lo:hi], start=(ec == 0), stop=(ec == EC - 1))
                hlo, hhi = lo // DK, hi // DK
                nc.vector.tensor_copy(
                    vsb[ni][:, hlo:hhi, 0:DK],
                    ps[:, 0:hi - lo].rearrange("p (h d) -> p h d", d=DK))
                if half == 1:
                    nc.vector.memset(vsb[ni][:, :, DK], 1.0)
            return (1.3, f)

        def av_unit(h, attns, g):
            def f():
                emit_av(h, attns, g)
            return (0.5, f)

        def pump(budget):
            while units and budget > 0:
                c, f = units.popleft()
                f()
                budget -= c

        # kt pair0 + qt pair0 emitted up front (head 0 critical path)
        emit_kt_half(0, 0)
        emit_kt_half(0, 1)
        pair_ready[0] = 2
        for mh in range(4):
            qt_unit(0, mh)[1]()
        for ni in range(NI):
            units.append(v_unit(ni, 0))
            units.append(v_unit(ni, 1))

        slot = 0
        for h in range(H):
            p = h // 2
            if h % 2 == 1 and p + 1 <= PAIRS - 1:
                # next pair's projections jump the queue so the even-head
                # boundary never has to force-drain a big batch
                for mh in range(3, -1, -1):
                    units.appendleft(qt_unit(p + 1, mh))
                units.appendleft(kt_unit(p + 1, 1))
                units.appendleft(kt_unit(p + 1, 0))
            while pair_ready.get(p, 0) < 6:
                c, f = units.popleft()
                f()
            attns = []
            for ni in range(NI):
                at = attn_pool.tile([128, M], BF16, tag="attn", name="attn_t")
                attns.append(at)
                for half in range(2):
                    emit_scores_exp_half(h, qts[h], ni, half, at)
                    pump(2.5 if slot < 12 else 0.75)
                slot += 1
            qts[h] = None  # allow qpt slot reuse
            for g in range(8):
                units.append(av_unit(h, attns, g))
        while units:
            c, f = units.popleft()
            f()

    nc.compile()
    return nc


def _get_nc():
    if "nc" not in _CACHE:
        _CACHE["nc"] = _build()
    return _CACHE["nc"]


def kernel(**inputs):
    global LAST_EXEC_NS, LAST_TRACE_DIR
    from concourse.bass_utils import run_bass_kernel_spmd

    ehr = np.asarray(inputs["ehr_embeddings"], dtype=np.float32)
    mi = np.asarray(inputs["missing_indices"]).astype(np.int64)
    ei = np.asarray(inputs["exist_indices"]).astype(np.int64)
    Wq = np.asarray(inputs["Wq"], dtype=np.float32)
    Wk = np.asarray(inputs["Wk"], dtype=np.float32)
    Wv = np.asarray(inputs["Wv"], dtype=np.float32)
    bq = np.asarray(inputs["bq"], dtype=np.float32)
    bv = np.asarray(inputs["bv"], dtype=np.float32)
    cooc = np.asarray(inputs["cooc_bias"], dtype=np.float32)
    # bk is softmax-shift-invariant (adds a per-query constant to scores);
    # dropped on device, consistent across cores so the combine is exact.

    scale = 1.0 / np.sqrt(np.float32(DK))
    bq_s = np.ascontiguousarray((bq * scale).reshape(PAIRS, 128).T)

    def fold(a):  # [E, F] -> [128, EC, F] (partition-major chunk fold)
        return np.ascontiguousarray(
            a.reshape(EC, 128, a.shape[1]).transpose(1, 0, 2))

    wq_s = fold((Wq * scale).astype(np.float16))
    wk_s = fold(Wk.astype(np.float16))
    wv_s = fold(Wv.astype(np.float16))
    missing_emb = ehr[mi]                       # [M, E]
    xt_m = fold(missing_emb.T.astype(np.float16))
    mbt = np.ascontiguousarray(
        cooc[:, mi, :].transpose(0, 2, 1).reshape(H * DK, M).astype(np.float16))

    common = {"xt_m": xt_m, "mbt": mbt, "wq": wq_s, "wk": wk_s, "wv": wv_s,
              "bq": bq_s}
    in_maps = []
    for c in range(CORES):
        eic = ei[c * NLOC:(c + 1) * NLOC]
        xt_e = fold(ehr[eic].T.astype(np.float16))  # [128, EC, NLOC]
        ebt = np.ascontiguousarray(
            cooc[:, eic, :].transpose(0, 2, 1).reshape(H * DK, NLOC).astype(np.float16))
        in_maps.append({**common, "xt_e": xt_e, "ebt": ebt})

    nc = _get_nc()
    trace = os.environ.get("KERNEL_TRACE") == "1"
    kwargs = {}
    if trace:
        import tempfile
        LAST_TRACE_DIR = tempfile.mkdtemp(prefix="kern_trace_")
        kwargs = {"trace": True, "tmpdir": LAST_TRACE_DIR}
        try:
            import ntff_shim
            ntff_shim.install()
        except ImportError:
            pass
    res = run_bass_kernel_spmd(nc, in_maps, list(range(CORES)), **kwargs)
    LAST_EXEC_NS = res.exec_time_ns

    # ---- host combine ----
    num = np.zeros((H, M, DK), dtype=np.float64)
    den = np.zeros((H, M), dtype=np.float64)
    for c in range(CORES):
        op = res.results[c]["out_p"].astype(np.float64)  # [H, M, DK+1]
        num += op[:, :, :DK]
        den += op[:, :, DK]
    out = num / den[:, :, None]                          # [H, M, DK]
    out = out.transpose(1, 0, 2).reshape(M, TOTAL) + bv.astype(np.float64)
    result = ehr.copy()
    result[mi] = out.astype(np.float32)
    return result


# revision 17
# speedup vs baseline: 1.0233x; 1.0233x over previous
"""MultiHeadSectionAttentionImputer on 8 TRN2 NeuronCores (Bass/Tile).

Sharding: the N=6144 existing sections are split across the 8 cores
(768 each). Each core:
  - projects its exist-shard to K,V  (K_loc = X_e @ Wk, V_loc = X_e @ Wv + ones col)
  - projects the full missing set to Q (duplicated across cores; Wq,bq
    pre-scaled by 1/sqrt(d_k) on host)
  - computes scoresT[n,m] per head with a fused 128-deep contraction:
      d' = [q-dims(64) | cooc-bias-dims(64)]  ->  q.k/sqrt(dk) + mb.eb
  - exp() without max subtraction (scores are bounded ~<60; fp32 range ok)
  - partial out^T = attn @ [V | 1]  ->  numerator (64 cols) + denominator
Host combines partial numerators/denominators across cores (softmax over
the full key set), adds bv, and scatters into a copy of ehr_embeddings.

All matmul inputs are float32r (tf32-like, full-rate on PE) except the
attention-weight matmul which uses bf16 (exp output cast, attn in [0, e^60]).
"""

import os
import sys
import numpy as np
from contextlib import ExitStack

sys.path.insert(0, "/opt/trn_rl_repo")

# problem constants (hardcoded; kernel.py must be self-contained)
H = 12          # heads
DK = 64         # head dim
E = 768         # embed dim
TOTAL = H * DK  # 768
M = 2048        # missing sections
N = 6144        # existing sections
S = 8192        # total sections
CORES = 8
NLOC = N // CORES        # 768 keys per core
EC = E // 128            # 6 contraction chunks
NI = NLOC // 128         # 6 key chunks per core
MI = M // 128            # 16 query chunks
PAIRS = H // 2           # 6 head pairs

_CACHE = {}
LAST_EXEC_NS = None
LAST_TRACE_DIR = None


def _build():
    import concourse.bass as bass
    import concourse.tile as tile
    from concourse import bacc, mybir

    F32 = mybir.dt.float32
    FP16 = mybir.dt.float16
    BF16 = mybir.dt.bfloat16
    Exp = mybir.ActivationFunctionType.Exp

    nc = bacc.Bacc("TRN2", target_bir_lowering=False, debug=False)

    # ---- I/O ----
    xt_m = nc.dram_tensor("xt_m", [128, 4, EC, 512], FP16, kind="ExternalInput").ap()
    mbt = nc.dram_tensor("mbt", [H * DK, M], FP16, kind="ExternalInput").ap()
    xt_ea = nc.dram_tensor("xt_ea", [128, EC, 512], FP16, kind="ExternalInput").ap()
    xt_eb = nc.dram_tensor("xt_eb", [128, EC, 256], FP16, kind="ExternalInput").ap()
    ebt = nc.dram_tensor("ebt", [H * DK, NLOC], FP16, kind="ExternalInput").ap()
    wq = nc.dram_tensor("wq", [128, PAIRS, EC, 128], FP16, kind="ExternalInput").ap()
    wk = nc.dram_tensor("wk", [128, PAIRS, EC, 128], FP16, kind="ExternalInput").ap()
    wv = nc.dram_tensor("wv", [128, EC, TOTAL], FP16, kind="ExternalInput").ap()
    bq = nc.dram_tensor("bq", [128, PAIRS], F32, kind="ExternalInput").ap()
    out_p = nc.dram_tensor("out_p", [H, M, DK + 1], F32, kind="ExternalOutput").ap()

    with tile.TileContext(nc) as tc, ExitStack() as ctx:
        persist = ctx.enter_context(tc.tile_pool(name="persist", bufs=1))
        qpt_pool = ctx.enter_context(tc.tile_pool(name="qpt", bufs=4))
        attn_pool = ctx.enter_context(tc.tile_pool(name="attn", bufs=18))
        osb_pool = ctx.enter_context(tc.tile_pool(name="osb", bufs=12))
        proj_ps = ctx.enter_context(tc.tile_pool(name="proj_ps", bufs=1, space="PSUM"))
        sc_ps = ctx.enter_context(tc.tile_pool(name="sc_ps", bufs=2, space="PSUM"))
        av_ps = ctx.enter_context(tc.tile_pool(name="av_ps", bufs=3, space="PSUM"))

        # input DMAs spread across queues, pieces ordered first-needed-first.
        # Only pair-0 column slices of wk/wq are on the critical path:
        #   sync:   wk[:,0:128] (kt0 lhsT), wk rest, xte col-rest
        #   scalar: bq, xte piece0 + xtm piece0 (kt0/qt0 rhs), xtm rest
        #   gpsimd: mbt h0/h1 + ebt p0 (q0/k0 bias rows), wq piece0, wv, wq rest
        kpt = [persist.tile([128, NLOC], FP16, tag=f"kpt{h}", name=f"kpt{h}") for h in range(H)]
        vsb = [persist.tile([128, H, DK + 1], BF16, tag=f"v{ni}", name=f"v{ni}") for ni in range(NI)]
        bq_sb = persist.tile([128, PAIRS], F32, tag="bq")
        nc.scalar.dma_start(bq_sb[:], bq)
        q0_0 = qpt_pool.tile([128, M], FP16, tag="qpt", name="qpt0")
        q1_0 = qpt_pool.tile([128, M], FP16, tag="qpt", name="qpt1")
        nc.gpsimd.dma_start(q0_0[64:128, :], mbt[0:DK, :])
        nc.gpsimd.dma_start(q1_0[0:64, :], mbt[DK:2 * DK, :])
        nc.gpsimd.dma_start(kpt[0][64:128, :], ebt[0:DK, :])
        nc.gpsimd.dma_start(kpt[1][0:64, :], ebt[DK:2 * DK, :])
        wk_big = persist.tile([128, PAIRS, EC, 128], FP16, tag="wk")
        wq_big = persist.tile([128, PAIRS, EC, 128], FP16, tag="wq")
        wv_big = persist.tile([128, EC, TOTAL], FP16, tag="wv")
        xtea_big = persist.tile([128, EC, 512], FP16, tag="xtea")
        xteb_big = persist.tile([128, EC, 256], FP16, tag="xteb")
        xtm_big = persist.tile([128, 4, EC, 512], FP16, tag="xtm")
        # contiguous-piece DMAs, critical (kt0/qt0) first
        nc.sync.dma_start(wk_big[:, 0], wk[:, 0])
        nc.scalar.dma_start(xtea_big[:], xt_ea)
        nc.scalar.dma_start(xtm_big[:, 0], xt_m[:, 0])
        nc.gpsimd.dma_start(wq_big[:, 0], wq[:, 0])
        nc.sync.dma_start(wk_big[:, 1:PAIRS], wk[:, 1:PAIRS])
        nc.scalar.dma_start(xtm_big[:, 1:4], xt_m[:, 1:4])
        nc.gpsimd.dma_start(wv_big[:], wv[:])
        nc.sync.dma_start(xteb_big[:], xt_eb)
        nc.gpsimd.dma_start(wq_big[:, 1:PAIRS], wq[:, 1:PAIRS])

        def emit_scores_exp_half(h, qt, ni, half, at):
            """scoresT half-chunk [128 keys, 1024 queries] + exp into attnT.
            Two halves -> the 2-bank scores psum double-buffers, keeping
            ACT busy back-to-back instead of waiting a full 4-matmul round."""
            ps = sc_ps.tile([128, 1024], F32, tag="sc", name="sc_ps_t")
            mo = half * 1024
            for mj in range(2):
                nc.tensor.matmul(
                    ps[:, mj * 512:(mj + 1) * 512],
                    lhsT=kpt[h][:, ni * 128:(ni + 1) * 128],
                    rhs=qt[:, mo + mj * 512:mo + (mj + 1) * 512],
                    start=True, stop=True)
            nc.scalar.activation(at[:, mo:mo + 1024], ps[:], Exp)

        def emit_av(h, attns, g):
            """out chunks [128 queries, DK+1] for head h, mi in [2g, 2g+2)."""
            for mi in range(2 * g, 2 * g + 2):
                ps = av_ps.tile([128, DK + 1], F32, tag="av", name="av_ps_t")
                for ni in range(NI):
                    nc.tensor.matmul(
                        ps[:], lhsT=attns[ni][:, mi * 128:(mi + 1) * 128],
                        rhs=vsb[ni][:, h, :],
                        start=(ni == 0), stop=(ni == NI - 1))
                ot = osb_pool.tile([128, DK + 1], F32, tag="osb", name="osb_t")
                nc.vector.tensor_copy(ot[:], ps[:])
                nc.sync.dma_start(out_p[h, mi * 128:(mi + 1) * 128, :], ot[:])

        # ---- emission schedule ----
        # Unit-queue of (pe_cost_us, fn): drained between scores/exp
        # emissions under a per-sub-slot budget so the PE fills the exp
        # pipeline gaps without pushing the next scores matmuls far back
        # in its (in-order) stream.
        from collections import deque
        units = deque()
        qts = {0: q0_0, 1: q1_0}
        pair_ready = {0: 0}  # pair -> emitted kt+qt piece count (6 = ready)

        def qt_unit(p, mh):
            def f():
                pair_ready[p] = pair_ready.get(p, 0) + 1
                q0, q1 = qts.get(2 * p), qts.get(2 * p + 1)
                if q0 is None:
                    q0 = qpt_pool.tile([128, M], FP16, tag="qpt", name=f"qpt{2*p}")
                    q1 = qpt_pool.tile([128, M], FP16, tag="qpt", name=f"qpt{2*p+1}")
                    h0, h1 = 2 * p, 2 * p + 1
                    nc.sync.dma_start(q0[64:128, :], mbt[h0 * DK:(h0 + 1) * DK, :])
                    nc.sync.dma_start(q1[0:64, :], mbt[h1 * DK:(h1 + 1) * DK, :])
                    qts[2 * p], qts[2 * p + 1] = q0, q1
                emit_qt_half(p, mh, q0, q1)
            return (1.3, f)

        def emit_qt_half(p, mh, q0, q1):
            ps = proj_ps.tile([128, 512], F32, tag="proj", name="proj_qt")
            for ec in range(EC):
                nc.tensor.matmul(ps[:], lhsT=wq_big[:, p, ec, :],
                                 rhs=xtm_big[:, mh, ec, :],
                                 start=(ec == 0), stop=(ec == EC - 1))
            mo = mh * 512
            nc.vector.tensor_scalar_add(
                q0[0:64, mo:mo + 512], ps[0:64, :], bq_sb[0:64, p:p + 1])
            nc.vector.tensor_scalar_add(
                q1[64:128, mo:mo + 512], ps[64:128, :], bq_sb[64:128, p:p + 1])

        def kt_unit(p, half):
            def f():
                pair_ready[p] = pair_ready.get(p, 0) + 1
                emit_kt_half(p, half)
            return (1.4, f)

        def emit_kt_half(p, half):
            h0, h1 = 2 * p, 2 * p + 1
            lo, hi = (0, 512) if half == 0 else (512, NLOC)
            ps = proj_ps.tile([128, 512], F32, tag="proj", name="proj_kt")
            xsrc = xtea_big if half == 0 else xteb_big
            for ec in range(EC):
                nc.tensor.matmul(ps[:, 0:hi - lo], lhsT=wk_big[:, p, ec, :],
                                 rhs=xsrc[:, ec, :], start=(ec == 0), stop=(ec == EC - 1))
            nc.vector.tensor_copy(kpt[h0][0:64, lo:hi], ps[0:64, 0:hi - lo])
            nc.vector.tensor_copy(kpt[h1][64:128, lo:hi], ps[64:128, 0:hi - lo])
            if half == 0 and p > 0:
                nc.sync.dma_start(kpt[h0][64:128, :], ebt[h0 * DK:(h0 + 1) * DK, :])
                nc.sync.dma_start(kpt[h1][0:64, :], ebt[h1 * DK:(h1 + 1) * DK, :])

        def v_unit(ni, half):
            def f():
                lo, hi = (0, 512) if half == 0 else (512, TOTAL)
                ps = proj_ps.tile([128, 512], F32, tag="proj", name="proj_v")
                for ec in range(EC):
                    xs = (xtea_big[:, ec, ni * 128:(ni + 1) * 128] if ni < 4
                          else xteb_big[:, ec, (ni - 4) * 128:(ni - 3) * 128])
                    nc.tensor.matmul(ps[:, 0:hi - lo], lhsT=xs,
                                     rhs=wv_big[:, ec, lo:hi], start=(ec == 0), stop=(ec == EC - 1))
                hlo, hhi = lo // DK, hi // DK
                nc.vector.tensor_copy(
                    vsb[ni][:, hlo:hhi, 0:DK],
                    ps[:, 0:hi - lo].rearrange("p (h d) -> p h d", d=DK))
                if half == 1:
                    nc.vector.memset(vsb[ni][:, :, DK], 1.0)
            return (1.3, f)

        def av_unit(h, attns, g):
            def f():
                emit_av(h, attns, g)
            return (0.5, f)

        def pump(budget):
            while units and budget > 0:
                c, f = units.popleft()
                f()
                budget -= c

        # kt pair0 + qt pair0 emitted up front (head 0 critical path)
        emit_kt_half(0, 0)
        emit_kt_half(0, 1)
        pair_ready[0] = 2
        for mh in range(4):
            qt_unit(0, mh)[1]()
        for ni in range(NI):
            units.append(v_unit(ni, 0))
            units.append(v_unit(ni, 1))

        slot = 0
        for h in range(H):
            p = h // 2
            if h % 2 == 1 and p + 1 <= PAIRS - 1:
                # next pair's projections jump the queue so the even-head
                # boundary never has to force-drain a big batch
                for mh in range(3, -1, -1):
                    units.appendleft(qt_unit(p + 1, mh))
                units.appendleft(kt_unit(p + 1, 1))
                units.appendleft(kt_unit(p + 1, 0))
            while pair_ready.get(p, 0) < 6:
                c, f = units.popleft()
                f()
            attns = []
            for ni in range(NI):
                at = attn_pool.tile([128, M], BF16, tag="attn", name="attn_t")
                attns.append(at)
                for half in range(2):
                    emit_scores_exp_half(h, qts[h], ni, half, at)
                    pump(2.5 if slot < 12 else 0.75)
                slot += 1
            qts[h] = None  # allow qpt slot reuse
            for g in range(8):
                units.append(av_unit(h, attns, g))
        while units:
            c, f = units.popleft()
            f()

    nc.compile()
    return nc


def _get_nc():
    if "nc" not in _CACHE:
        _CACHE["nc"] = _build()
    return _CACHE["nc"]


def kernel(**inputs):
    global LAST_EXEC_NS, LAST_TRACE_DIR
    from concourse.bass_utils import run_bass_kernel_spmd

    ehr = np.asarray(inputs["ehr_embeddings"], dtype=np.float32)
    mi = np.asarray(inputs["missing_indices"]).astype(np.int64)
    ei = np.asarray(inputs["exist_indices"]).astype(np.int64)
    Wq = np.asarray(inputs["Wq"], dtype=np.float32)
    Wk = np.asarray(inputs["Wk"], dtype=np.float32)
    Wv = np.asarray(inputs["Wv"], dtype=np.float32)
    bq = np.asarray(inputs["bq"], dtype=np.float32)
    bv = np.asarray(inputs["bv"], dtype=np.float32)
    cooc = np.asarray(inputs["cooc_bias"], dtype=np.float32)
    # bk is softmax-shift-invariant (adds a per-query constant to scores);
    # dropped on device, consistent across cores so the combine is exact.

    scale = 1.0 / np.sqrt(np.float32(DK))
    bq_s = np.ascontiguousarray((bq * scale).reshape(PAIRS, 128).T)

    def fold(a):  # [E, F] -> [128, EC, F] (partition-major chunk fold)
        return a.reshape(EC, 128, a.shape[1]).transpose(1, 0, 2)

    def wfold(a):  # [E, TOTAL] -> [128, PAIRS, EC, 128] (pair-col major)
        return np.ascontiguousarray(
            fold(a).reshape(128, EC, PAIRS, 128).transpose(0, 2, 1, 3))

    wq_s = wfold((Wq * scale).astype(np.float16))
    wk_s = wfold(Wk.astype(np.float16))
    wv_s = np.ascontiguousarray(fold(Wv.astype(np.float16)))
    missing_emb = ehr[mi]                       # [M, E]
    xt_m = np.ascontiguousarray(
        fold(missing_emb.T.astype(np.float16))
        .reshape(128, EC, 4, 512).transpose(0, 2, 1, 3))  # [128, 4, EC, 512]
    mbt = np.ascontiguousarray(
        cooc[:, mi, :].transpose(0, 2, 1).reshape(H * DK, M).astype(np.float16))

    common = {"xt_m": xt_m, "mbt": mbt, "wq": wq_s, "wk": wk_s, "wv": wv_s,
              "bq": bq_s}
    in_maps = []
    for c in range(CORES):
        eic = ei[c * NLOC:(c + 1) * NLOC]
        xte_f = fold(ehr[eic].T.astype(np.float16))  # [128, EC, NLOC]
        ebt = np.ascontiguousarray(
            cooc[:, eic, :].transpose(0, 2, 1).reshape(H * DK, NLOC).astype(np.float16))
        in_maps.append({**common, "xt_ea": np.ascontiguousarray(xte_f[:, :, 0:512]),
                        "xt_eb": np.ascontiguousarray(xte_f[:, :, 512:NLOC]),
                        "ebt": ebt})

    nc = _get_nc()
    trace = os.environ.get("KERNEL_TRACE") == "1"
    kwargs = {}
    if trace:
        import tempfile
        LAST_TRACE_DIR = tempfile.mkdtemp(prefix="kern_trace_")
        kwargs = {"trace": True, "tmpdir": LAST_TRACE_DIR}
        try:
            import ntff_shim
            ntff_shim.install()
        except ImportError:
            pass
    res = run_bass_kernel_spmd(nc, in_maps, list(range(CORES)), **kwargs)
    LAST_EXEC_NS = res.exec_time_ns

    # ---- host combine ----
    num = np.zeros((H, M, DK), dtype=np.float64)
    den = np.zeros((H, M), dtype=np.float64)
    for c in range(CORES):
        op = res.results[c]["out_p"].astype(np.float64)  # [H, M, DK+1]
        num += op[:, :, :DK]
        den += op[:, :, DK]
    out = num / den[:, :, None]                          # [H, M, DK]
    out = out.transpose(1, 0, 2).reshape(M, TOTAL) + bv.astype(np.float64)
    result = ehr.copy()
    result[mi] = out.astype(np.float32)
    return result


# revision 18
# speedup vs baseline: 1.0637x; 1.0395x over previous
"""MultiHeadSectionAttentionImputer on 8 TRN2 NeuronCores (Bass/Tile).

Sharding: the N=6144 existing sections are split across the 8 cores
(768 each). Each core:
  - projects its exist-shard to K,V  (K_loc = X_e @ Wk, V_loc = X_e @ Wv + ones col)
  - projects the full missing set to Q (duplicated across cores; Wq,bq
    pre-scaled by 1/sqrt(d_k) on host)
  - computes scoresT[n,m] per head with a fused 128-deep contraction:
      d' = [q-dims(64) | cooc-bias-dims(64)]  ->  q.k/sqrt(dk) + mb.eb
  - exp() without max subtraction (scores are bounded ~<60; fp32 range ok)
  - partial out^T = attn @ [V | 1]  ->  numerator (64 cols) + denominator
Host combines partial numerators/denominators across cores (softmax over
the full key set), adds bv, and scatters into a copy of ehr_embeddings.

All matmul inputs are float32r (tf32-like, full-rate on PE) except the
attention-weight matmul which uses bf16 (exp output cast, attn in [0, e^60]).
"""

import os
import sys
import numpy as np
from contextlib import ExitStack

sys.path.insert(0, "/opt/trn_rl_repo")

# problem constants (hardcoded; kernel.py must be self-contained)
H = 12          # heads
DK = 64         # head dim
E = 768         # embed dim
TOTAL = H * DK  # 768
M = 2048        # missing sections
N = 6144        # existing sections
S = 8192        # total sections
CORES = 8
NLOC = N // CORES        # 768 keys per core
EC = E // 128            # 6 contraction chunks
NI = NLOC // 128         # 6 key chunks per core
MI = M // 128            # 16 query chunks
PAIRS = H // 2           # 6 head pairs

_CACHE = {}
LAST_EXEC_NS = None
LAST_TRACE_DIR = None


def _build():
    import concourse.bass as bass
    import concourse.tile as tile
    from concourse import bacc, mybir

    F32 = mybir.dt.float32
    FP16 = mybir.dt.float16
    BF16 = mybir.dt.bfloat16
    Exp = mybir.ActivationFunctionType.Exp

    nc = bacc.Bacc("TRN2", target_bir_lowering=False, debug=False)

    # ---- I/O ----
    xt_m = nc.dram_tensor("xt_m", [128, 4, EC, 512], FP16, kind="ExternalInput").ap()
    mbt = nc.dram_tensor("mbt", [H * DK, M], FP16, kind="ExternalInput").ap()
    xt_ea = nc.dram_tensor("xt_ea", [128, EC, 512], FP16, kind="ExternalInput").ap()
    xt_eb = nc.dram_tensor("xt_eb", [128, EC, 256], FP16, kind="ExternalInput").ap()
    ebt = nc.dram_tensor("ebt", [H * DK, NLOC], FP16, kind="ExternalInput").ap()
    wq = nc.dram_tensor("wq", [128, PAIRS, EC, 128], FP16, kind="ExternalInput").ap()
    wk = nc.dram_tensor("wk", [128, PAIRS, EC, 128], FP16, kind="ExternalInput").ap()
    wv = nc.dram_tensor("wv", [128, EC, TOTAL], FP16, kind="ExternalInput").ap()
    bq = nc.dram_tensor("bq", [128, PAIRS], F32, kind="ExternalInput").ap()
    out_p = nc.dram_tensor("out_p", [H, M, DK + 1], F32, kind="ExternalOutput").ap()

    with tile.TileContext(nc) as tc, ExitStack() as ctx:
        persist = ctx.enter_context(tc.tile_pool(name="persist", bufs=1))
        qpt_pool = ctx.enter_context(tc.tile_pool(name="qpt", bufs=4))
        attn_pool = ctx.enter_context(tc.tile_pool(name="attn", bufs=18))
        osb_pool = ctx.enter_context(tc.tile_pool(name="osb", bufs=12))
        proj_ps = ctx.enter_context(tc.tile_pool(name="proj_ps", bufs=1, space="PSUM"))
        sc_ps = ctx.enter_context(tc.tile_pool(name="sc_ps", bufs=2, space="PSUM"))
        av_ps = ctx.enter_context(tc.tile_pool(name="av_ps", bufs=3, space="PSUM"))

        # input DMAs spread across queues, pieces ordered first-needed-first.
        # Only pair-0 column slices of wk/wq are on the critical path:
        #   sync:   wk[:,0:128] (kt0 lhsT), wk rest, xte col-rest
        #   scalar: bq, xte piece0 + xtm piece0 (kt0/qt0 rhs), xtm rest
        #   gpsimd: mbt h0/h1 + ebt p0 (q0/k0 bias rows), wq piece0, wv, wq rest
        kpt = [persist.tile([128, NLOC], FP16, tag=f"kpt{h}", name=f"kpt{h}") for h in range(H)]
        vsb = [persist.tile([128, H, DK + 1], BF16, tag=f"v{ni}", name=f"v{ni}") for ni in range(NI)]
        bq_sb = persist.tile([128, PAIRS], F32, tag="bq")
        nc.scalar.dma_start(bq_sb[:], bq)
        q0_0 = qpt_pool.tile([128, M], FP16, tag="qpt", name="qpt0")
        q1_0 = qpt_pool.tile([128, M], FP16, tag="qpt", name="qpt1")
        nc.gpsimd.dma_start(q0_0[64:128, :], mbt[0:DK, :])
        nc.gpsimd.dma_start(q1_0[0:64, :], mbt[DK:2 * DK, :])
        nc.gpsimd.dma_start(kpt[0][64:128, :], ebt[0:DK, :])
        nc.gpsimd.dma_start(kpt[1][0:64, :], ebt[DK:2 * DK, :])
        wk_big = persist.tile([128, PAIRS, EC, 128], FP16, tag="wk")
        wq_big = persist.tile([128, PAIRS, EC, 128], FP16, tag="wq")
        wv_big = persist.tile([128, EC, TOTAL], FP16, tag="wv")
        xtea_big = persist.tile([128, EC, 512], FP16, tag="xtea")
        xteb_big = persist.tile([128, EC, 256], FP16, tag="xteb")
        xtm_big = persist.tile([128, 4, EC, 512], FP16, tag="xtm")
        # contiguous-piece DMAs, critical (kt0/qt0) first
        nc.sync.dma_start(wk_big[:, 0], wk[:, 0])
        nc.scalar.dma_start(xtea_big[:], xt_ea)
        nc.scalar.dma_start(xtm_big[:, 0], xt_m[:, 0])
        nc.gpsimd.dma_start(wq_big[:, 0], wq[:, 0])
        nc.sync.dma_start(wk_big[:, 1:PAIRS], wk[:, 1:PAIRS])
        nc.scalar.dma_start(xtm_big[:, 1:4], xt_m[:, 1:4])
        nc.gpsimd.dma_start(wv_big[:], wv[:])
        nc.sync.dma_start(xteb_big[:], xt_eb)
        nc.gpsimd.dma_start(wq_big[:, 1:PAIRS], wq[:, 1:PAIRS])

        def emit_scores_exp_half(h, qt, ni, half, at):
            """scoresT half-chunk [128 keys, 1024 queries] + exp into attnT.
            Two halves -> the 2-bank scores psum double-buffers, keeping
            ACT busy back-to-back instead of waiting a full 4-matmul round."""
            ps = sc_ps.tile([128, 1024], F32, tag="sc", name="sc_ps_t")
            mo = half * 1024
            for mj in range(2):
                nc.tensor.matmul(
                    ps[:, mj * 512:(mj + 1) * 512],
                    lhsT=kpt[h][:, ni * 128:(ni + 1) * 128],
                    rhs=qt[:, mo + mj * 512:mo + (mj + 1) * 512],
                    start=True, stop=True)
            nc.scalar.activation(at[:, mo:mo + 1024], ps[:], Exp)

        def emit_av(h, attns, g):
            """out chunks [128 queries, DK+1] for head h, mi in [2g, 2g+2)."""
            for mi in range(2 * g, 2 * g + 2):
                ps = av_ps.tile([128, DK + 1], F32, tag="av", name="av_ps_t")
                for ni in range(NI):
                    nc.tensor.matmul(
                        ps[:], lhsT=attns[ni][:, mi * 128:(mi + 1) * 128],
                        rhs=vsb[ni][:, h, :],
                        start=(ni == 0), stop=(ni == NI - 1))
                ot = osb_pool.tile([128, DK + 1], F32, tag="osb", name="osb_t")
                nc.vector.tensor_copy(ot[:], ps[:])
                nc.sync.dma_start(out_p[h, mi * 128:(mi + 1) * 128, :], ot[:])

        # ---- emission schedule ----
        # Unit-queue of (pe_cost_us, fn): drained between scores/exp
        # emissions under a per-sub-slot budget so the PE fills the exp
        # pipeline gaps without pushing the next scores matmuls far back
        # in its (in-order) stream.
        from collections import deque
        units = deque()
        qts = {0: q0_0, 1: q1_0}
        pieces = {0: set()}  # pair -> done piece ids (k0,k1,q0..q3)

        def qt_unit(p, mh):
            def f():
                pieces.setdefault(p, set()).add(f"q{mh}")
                q0, q1 = qts.get(2 * p), qts.get(2 * p + 1)
                if q0 is None:
                    q0 = qpt_pool.tile([128, M], FP16, tag="qpt", name=f"qpt{2*p}")
                    q1 = qpt_pool.tile([128, M], FP16, tag="qpt", name=f"qpt{2*p+1}")
                    h0, h1 = 2 * p, 2 * p + 1
                    nc.sync.dma_start(q0[64:128, :], mbt[h0 * DK:(h0 + 1) * DK, :])
                    nc.sync.dma_start(q1[0:64, :], mbt[h1 * DK:(h1 + 1) * DK, :])
                    qts[2 * p], qts[2 * p + 1] = q0, q1
                emit_qt_half(p, mh, q0, q1)
            return (1.3, f)

        def emit_qt_half(p, mh, q0, q1):
            ps = proj_ps.tile([128, 512], F32, tag="proj", name="proj_qt")
            for ec in range(EC):
                nc.tensor.matmul(ps[:], lhsT=wq_big[:, p, ec, :],
                                 rhs=xtm_big[:, mh, ec, :],
                                 start=(ec == 0), stop=(ec == EC - 1))
            mo = mh * 512
            nc.vector.tensor_scalar_add(
                q0[0:64, mo:mo + 512], ps[0:64, :], bq_sb[0:64, p:p + 1])
            nc.vector.tensor_scalar_add(
                q1[64:128, mo:mo + 512], ps[64:128, :], bq_sb[64:128, p:p + 1])

        def kt_unit(p, half):
            def f():
                pieces.setdefault(p, set()).add(f"k{half}")
                emit_kt_half(p, half)
            return (1.4, f)

        def emit_kt_half(p, half):
            h0, h1 = 2 * p, 2 * p + 1
            lo, hi = (0, 512) if half == 0 else (512, NLOC)
            ps = proj_ps.tile([128, 512], F32, tag="proj", name="proj_kt")
            xsrc = xtea_big if half == 0 else xteb_big
            for ec in range(EC):
                nc.tensor.matmul(ps[:, 0:hi - lo], lhsT=wk_big[:, p, ec, :],
                                 rhs=xsrc[:, ec, :], start=(ec == 0), stop=(ec == EC - 1))
            nc.vector.tensor_copy(kpt[h0][0:64, lo:hi], ps[0:64, 0:hi - lo])
            nc.vector.tensor_copy(kpt[h1][64:128, lo:hi], ps[64:128, 0:hi - lo])
            if half == 0 and p > 0:
                nc.sync.dma_start(kpt[h0][64:128, :], ebt[h0 * DK:(h0 + 1) * DK, :])
                nc.sync.dma_start(kpt[h1][0:64, :], ebt[h1 * DK:(h1 + 1) * DK, :])

        def v_unit(ni, half):
            def f():
                lo, hi = (0, 512) if half == 0 else (512, TOTAL)
                ps = proj_ps.tile([128, 512], F32, tag="proj", name="proj_v")
                for ec in range(EC):
                    xs = (xtea_big[:, ec, ni * 128:(ni + 1) * 128] if ni < 4
                          else xteb_big[:, ec, (ni - 4) * 128:(ni - 3) * 128])
                    nc.tensor.matmul(ps[:, 0:hi - lo], lhsT=xs,
                                     rhs=wv_big[:, ec, lo:hi], start=(ec == 0), stop=(ec == EC - 1))
                hlo, hhi = lo // DK, hi // DK
                nc.vector.tensor_copy(
                    vsb[ni][:, hlo:hhi, 0:DK],
                    ps[:, 0:hi - lo].rearrange("p (h d) -> p h d", d=DK))
                if half == 1:
                    nc.vector.memset(vsb[ni][:, :, DK], 1.0)
            return (1.3, f)

        def av_unit(h, attns, g):
            def f():
                emit_av(h, attns, g)
            return (0.5, f)

        def pump(budget):
            while units and budget > 0:
                c, f = units.popleft()
                f()
                budget -= c

        # minimal head-0 critical path up front: kt half0 + qt q0/q1 only
        emit_kt_half(0, 0)
        pieces[0].add("k0")
        qt_unit(0, 0)[1]()
        qt_unit(0, 1)[1]()
        units.append(kt_unit(0, 1))
        units.append(qt_unit(0, 2))
        units.append(qt_unit(0, 3))
        for ni in range(NI):
            units.append(v_unit(ni, 0))
            units.append(v_unit(ni, 1))

        def need(p, ni, half):
            req = {"k0" if ni < 4 else "k1", f"q{2 * half}", f"q{2 * half + 1}"}
            while not req <= pieces.get(p, set()):
                c, f = units.popleft()
                f()

        slot = 0
        for h in range(H):
            p = h // 2
            if h % 2 == 1 and p + 1 <= PAIRS - 1:
                # next pair's projections jump the queue so the even-head
                # boundary never has to force-drain a big batch
                for mh in range(3, -1, -1):
                    units.appendleft(qt_unit(p + 1, mh))
                units.appendleft(kt_unit(p + 1, 1))
                units.appendleft(kt_unit(p + 1, 0))
            attns = [attn_pool.tile([128, M], BF16, tag="attn", name=f"attn_t{h}_{ni}")
                     for ni in range(NI)]
            for half in range(2):
                for ni in range(NI):
                    need(p, ni, half)
                    emit_scores_exp_half(h, qts[h], ni, half, attns[ni])
                    pump(2.5 if slot < 12 else 0.75)
                    slot += 1
            qts[h] = None  # allow qpt slot reuse
            for g in range(8):
                units.append(av_unit(h, attns, g))
        while units:
            c, f = units.popleft()
            f()

    nc.compile()
    return nc


def _get_nc():
    if "nc" not in _CACHE:
        _CACHE["nc"] = _build()
    return _CACHE["nc"]


def kernel(**inputs):
    global LAST_EXEC_NS, LAST_TRACE_DIR
    from concourse.bass_utils import run_bass_kernel_spmd

    ehr = np.asarray(inputs["ehr_embeddings"], dtype=np.float32)
    mi = np.asarray(inputs["missing_indices"]).astype(np.int64)
    ei = np.asarray(inputs["exist_indices"]).astype(np.int64)
    Wq = np.asarray(inputs["Wq"], dtype=np.float32)
    Wk = np.asarray(inputs["Wk"], dtype=np.float32)
    Wv = np.asarray(inputs["Wv"], dtype=np.float32)
    bq = np.asarray(inputs["bq"], dtype=np.float32)
    bv = np.asarray(inputs["bv"], dtype=np.float32)
    cooc = np.asarray(inputs["cooc_bias"], dtype=np.float32)
    # bk is softmax-shift-invariant (adds a per-query constant to scores);
    # dropped on device, consistent across cores so the combine is exact.

    scale = 1.0 / np.sqrt(np.float32(DK))
    bq_s = np.ascontiguousarray((bq * scale).reshape(PAIRS, 128).T)

    def fold(a):  # [E, F] -> [128, EC, F] (partition-major chunk fold)
        return a.reshape(EC, 128, a.shape[1]).transpose(1, 0, 2)

    def wfold(a):  # [E, TOTAL] -> [128, PAIRS, EC, 128] (pair-col major)
        return np.ascontiguousarray(
            fold(a).reshape(128, EC, PAIRS, 128).transpose(0, 2, 1, 3))

    wq_s = wfold((Wq * scale).astype(np.float16))
    wk_s = wfold(Wk.astype(np.float16))
    wv_s = np.ascontiguousarray(fold(Wv.astype(np.float16)))
    missing_emb = ehr[mi]                       # [M, E]
    xt_m = np.ascontiguousarray(
        fold(missing_emb.T.astype(np.float16))
        .reshape(128, EC, 4, 512).transpose(0, 2, 1, 3))  # [128, 4, EC, 512]
    mbt = np.ascontiguousarray(
        cooc[:, mi, :].transpose(0, 2, 1).reshape(H * DK, M).astype(np.float16))

    common = {"xt_m": xt_m, "mbt": mbt, "wq": wq_s, "wk": wk_s, "wv": wv_s,
              "bq": bq_s}
    in_maps = []
    for c in range(CORES):
        eic = ei[c * NLOC:(c + 1) * NLOC]
        xte_f = fold(ehr[eic].T.astype(np.float16))  # [128, EC, NLOC]
        ebt = np.ascontiguousarray(
            cooc[:, eic, :].transpose(0, 2, 1).reshape(H * DK, NLOC).astype(np.float16))
        in_maps.append({**common, "xt_ea": np.ascontiguousarray(xte_f[:, :, 0:512]),
                        "xt_eb": np.ascontiguousarray(xte_f[:, :, 512:NLOC]),
                        "ebt": ebt})

    nc = _get_nc()
    trace = os.environ.get("KERNEL_TRACE") == "1"
    kwargs = {}
    if trace:
        import tempfile
        LAST_TRACE_DIR = tempfile.mkdtemp(prefix="kern_trace_")
        kwargs = {"trace": True, "tmpdir": LAST_TRACE_DIR}
        try:
            import ntff_shim
            ntff_shim.install()
        except ImportError:
            pass
    res = run_bass_kernel_spmd(nc, in_maps, list(range(CORES)), **kwargs)
    LAST_EXEC_NS = res.exec_time_ns

    # ---- host combine ----
    num = np.zeros((H, M, DK), dtype=np.float64)
    den = np.zeros((H, M), dtype=np.float64)
    for c in range(CORES):
        op = res.results[c]["out_p"].astype(np.float64)  # [H, M, DK+1]
        num += op[:, :, :DK]
        den += op[:, :, DK]
    out = num / den[:, :, None]                          # [H, M, DK]
    out = out.transpose(1, 0, 2).reshape(M, TOTAL) + bv.astype(np.float64)
    result = ehr.copy()
    result[mi] = out.astype(np.float32)
    return result


# revision 19
# speedup vs baseline: 1.0700x; 1.0059x over previous
"""MultiHeadSectionAttentionImputer on 8 TRN2 NeuronCores (Bass/Tile).

Sharding: the N=6144 existing sections are split across the 8 cores
(768 each). Each core:
  - projects its exist-shard to K,V  (K_loc = X_e @ Wk, V_loc = X_e @ Wv + ones col)
  - projects the full missing set to Q (duplicated across cores; Wq,bq
    pre-scaled by 1/sqrt(d_k) on host)
  - computes scoresT[n,m] per head with a fused 128-deep contraction:
      d' = [q-dims(64) | cooc-bias-dims(64)]  ->  q.k/sqrt(dk) + mb.eb
  - exp() without max subtraction (scores are bounded ~<60; fp32 range ok)
  - partial out^T = attn @ [V | 1]  ->  numerator (64 cols) + denominator
Host combines partial numerators/denominators across cores (softmax over
the full key set), adds bv, and scatters into a copy of ehr_embeddings.

All matmul inputs are float32r (tf32-like, full-rate on PE) except the
attention-weight matmul which uses bf16 (exp output cast, attn in [0, e^60]).
"""

import os
import sys
import numpy as np
from contextlib import ExitStack

sys.path.insert(0, "/opt/trn_rl_repo")

# problem constants (hardcoded; kernel.py must be self-contained)
H = 12          # heads
DK = 64         # head dim
E = 768         # embed dim
TOTAL = H * DK  # 768
M = 2048        # missing sections
N = 6144        # existing sections
S = 8192        # total sections
CORES = 8
NLOC = N // CORES        # 768 keys per core
EC = E // 128            # 6 contraction chunks
NI = NLOC // 128         # 6 key chunks per core
MI = M // 128            # 16 query chunks
PAIRS = H // 2           # 6 head pairs

_CACHE = {}
LAST_EXEC_NS = None
LAST_TRACE_DIR = None


def _build():
    import concourse.bass as bass
    import concourse.tile as tile
    from concourse import bacc, mybir

    F32 = mybir.dt.float32
    FP16 = mybir.dt.float16
    BF16 = mybir.dt.bfloat16
    Exp = mybir.ActivationFunctionType.Exp

    nc = bacc.Bacc("TRN2", target_bir_lowering=False, debug=False)

    # ---- I/O ----
    xt_m = nc.dram_tensor("xt_m", [128, 4, EC, 512], FP16, kind="ExternalInput").ap()
    mbt = nc.dram_tensor("mbt", [H * DK, M], FP16, kind="ExternalInput").ap()
    xt_ea = nc.dram_tensor("xt_ea", [128, EC, 512], FP16, kind="ExternalInput").ap()
    xt_eb = nc.dram_tensor("xt_eb", [128, EC, 256], FP16, kind="ExternalInput").ap()
    ebt = nc.dram_tensor("ebt", [H * DK, NLOC], FP16, kind="ExternalInput").ap()
    wq = nc.dram_tensor("wq", [128, PAIRS, EC, 128], FP16, kind="ExternalInput").ap()
    wk = nc.dram_tensor("wk", [128, PAIRS, EC, 128], FP16, kind="ExternalInput").ap()
    wv = nc.dram_tensor("wv", [128, EC, TOTAL], FP16, kind="ExternalInput").ap()
    bq = nc.dram_tensor("bq", [128, PAIRS], F32, kind="ExternalInput").ap()
    out_p = nc.dram_tensor("out_p", [H, M, DK + 1], F32, kind="ExternalOutput").ap()

    with tile.TileContext(nc) as tc, ExitStack() as ctx:
        persist = ctx.enter_context(tc.tile_pool(name="persist", bufs=1))
        qpt_pool = ctx.enter_context(tc.tile_pool(name="qpt", bufs=4))
        attn_pool = ctx.enter_context(tc.tile_pool(name="attn", bufs=18))
        osb_pool = ctx.enter_context(tc.tile_pool(name="osb", bufs=12))
        proj_ps = ctx.enter_context(tc.tile_pool(name="proj_ps", bufs=1, space="PSUM"))
        sc_ps = ctx.enter_context(tc.tile_pool(name="sc_ps", bufs=2, space="PSUM"))
        av_ps = ctx.enter_context(tc.tile_pool(name="av_ps", bufs=3, space="PSUM"))

        # input DMAs spread across queues, pieces ordered first-needed-first.
        # Only pair-0 column slices of wk/wq are on the critical path:
        #   sync:   wk[:,0:128] (kt0 lhsT), wk rest, xte col-rest
        #   scalar: bq, xte piece0 + xtm piece0 (kt0/qt0 rhs), xtm rest
        #   gpsimd: mbt h0/h1 + ebt p0 (q0/k0 bias rows), wq piece0, wv, wq rest
        kpt = [persist.tile([128, NLOC], FP16, tag=f"kpt{h}", name=f"kpt{h}") for h in range(H)]
        vsb = [persist.tile([128, H, DK + 1], BF16, tag=f"v{ni}", name=f"v{ni}") for ni in range(NI)]
        bq_sb = persist.tile([128, PAIRS], F32, tag="bq")
        nc.scalar.dma_start(bq_sb[:], bq)
        q0_0 = qpt_pool.tile([128, M], FP16, tag="qpt", name="qpt0")
        q1_0 = qpt_pool.tile([128, M], FP16, tag="qpt", name="qpt1")
        nc.gpsimd.dma_start(q0_0[64:128, :], mbt[0:DK, :])
        nc.gpsimd.dma_start(q1_0[0:64, :], mbt[DK:2 * DK, :])
        nc.gpsimd.dma_start(kpt[0][64:128, :], ebt[0:DK, :])
        nc.gpsimd.dma_start(kpt[1][0:64, :], ebt[DK:2 * DK, :])
        wk_big = persist.tile([128, PAIRS, EC, 128], FP16, tag="wk")
        wq_big = persist.tile([128, PAIRS, EC, 128], FP16, tag="wq")
        wv_big = persist.tile([128, EC, TOTAL], FP16, tag="wv")
        xtea_big = persist.tile([128, EC, 512], FP16, tag="xtea")
        xteb_big = persist.tile([128, EC, 256], FP16, tag="xteb")
        xtm_big = persist.tile([128, 4, EC, 512], FP16, tag="xtm")
        # contiguous-piece DMAs; criticals get dedicated queues, bulk
        # remainder rides gpsimd (SWDGE) which it doesn't gate anything on
        nc.sync.dma_start(wk_big[:, 0], wk[:, 0])
        nc.sync.dma_start(xteb_big[:], xt_eb)
        nc.scalar.dma_start(xtea_big[:], xt_ea)
        nc.scalar.dma_start(xtm_big[:, 0], xt_m[:, 0])
        nc.gpsimd.dma_start(wq_big[:, 0], wq[:, 0])
        nc.gpsimd.dma_start(wv_big[:], wv[:])
        nc.gpsimd.dma_start(wk_big[:, 1:PAIRS], wk[:, 1:PAIRS])
        nc.scalar.dma_start(xtm_big[:, 1:4], xt_m[:, 1:4])
        nc.gpsimd.dma_start(wq_big[:, 1:PAIRS], wq[:, 1:PAIRS])

        def emit_scores_exp_half(h, qt, ni, half, at):
            """scoresT half-chunk [128 keys, 1024 queries] + exp into attnT.
            Two halves -> the 2-bank scores psum double-buffers, keeping
            ACT busy back-to-back instead of waiting a full 4-matmul round."""
            ps = sc_ps.tile([128, 1024], F32, tag="sc", name="sc_ps_t")
            mo = half * 1024
            for mj in range(2):
                nc.tensor.matmul(
                    ps[:, mj * 512:(mj + 1) * 512],
                    lhsT=kpt[h][:, ni * 128:(ni + 1) * 128],
                    rhs=qt[:, mo + mj * 512:mo + (mj + 1) * 512],
                    start=True, stop=True)
            nc.scalar.activation(at[:, mo:mo + 1024], ps[:], Exp)

        def emit_av(h, attns, g):
            """out chunks [128 queries, DK+1] for head h, mi in [2g, 2g+2)."""
            for mi in range(2 * g, 2 * g + 2):
                ps = av_ps.tile([128, DK + 1], F32, tag="av", name="av_ps_t")
                for ni in range(NI):
                    nc.tensor.matmul(
                        ps[:], lhsT=attns[ni][:, mi * 128:(mi + 1) * 128],
                        rhs=vsb[ni][:, h, :],
                        start=(ni == 0), stop=(ni == NI - 1))
                ot = osb_pool.tile([128, DK + 1], F32, tag="osb", name="osb_t")
                nc.vector.tensor_copy(ot[:], ps[:])
                nc.sync.dma_start(out_p[h, mi * 128:(mi + 1) * 128, :], ot[:])

        # ---- emission schedule ----
        # Unit-queue of (pe_cost_us, fn): drained between scores/exp
        # emissions under a per-sub-slot budget so the PE fills the exp
        # pipeline gaps without pushing the next scores matmuls far back
        # in its (in-order) stream.
        from collections import deque
        units = deque()
        qts = {0: q0_0, 1: q1_0}
        pieces = {0: set()}  # pair -> done piece ids (k0,k1,q0..q3)

        def qt_unit(p, mh):
            def f():
                pieces.setdefault(p, set()).add(f"q{mh}")
                q0, q1 = qts.get(2 * p), qts.get(2 * p + 1)
                if q0 is None:
                    q0 = qpt_pool.tile([128, M], FP16, tag="qpt", name=f"qpt{2*p}")
                    q1 = qpt_pool.tile([128, M], FP16, tag="qpt", name=f"qpt{2*p+1}")
                    h0, h1 = 2 * p, 2 * p + 1
                    nc.sync.dma_start(q0[64:128, :], mbt[h0 * DK:(h0 + 1) * DK, :])
                    nc.sync.dma_start(q1[0:64, :], mbt[h1 * DK:(h1 + 1) * DK, :])
                    qts[2 * p], qts[2 * p + 1] = q0, q1
                emit_qt_half(p, mh, q0, q1)
            return (1.3, f)

        def emit_qt_half(p, mh, q0, q1):
            ps = proj_ps.tile([128, 512], F32, tag="proj", name="proj_qt")
            for ec in range(EC):
                nc.tensor.matmul(ps[:], lhsT=wq_big[:, p, ec, :],
                                 rhs=xtm_big[:, mh, ec, :],
                                 start=(ec == 0), stop=(ec == EC - 1))
            mo = mh * 512
            nc.vector.tensor_scalar_add(
                q0[0:64, mo:mo + 512], ps[0:64, :], bq_sb[0:64, p:p + 1])
            nc.vector.tensor_scalar_add(
                q1[64:128, mo:mo + 512], ps[64:128, :], bq_sb[64:128, p:p + 1])

        def kt_unit(p, half):
            def f():
                pieces.setdefault(p, set()).add(f"k{half}")
                emit_kt_half(p, half)
            return (1.4, f)

        def emit_kt_half(p, half):
            h0, h1 = 2 * p, 2 * p + 1
            lo, hi = (0, 512) if half == 0 else (512, NLOC)
            ps = proj_ps.tile([128, 512], F32, tag="proj", name="proj_kt")
            xsrc = xtea_big if half == 0 else xteb_big
            for ec in range(EC):
                nc.tensor.matmul(ps[:, 0:hi - lo], lhsT=wk_big[:, p, ec, :],
                                 rhs=xsrc[:, ec, :], start=(ec == 0), stop=(ec == EC - 1))
            nc.vector.tensor_copy(kpt[h0][0:64, lo:hi], ps[0:64, 0:hi - lo])
            nc.vector.tensor_copy(kpt[h1][64:128, lo:hi], ps[64:128, 0:hi - lo])
            if half == 0 and p > 0:
                nc.sync.dma_start(kpt[h0][64:128, :], ebt[h0 * DK:(h0 + 1) * DK, :])
                nc.sync.dma_start(kpt[h1][0:64, :], ebt[h1 * DK:(h1 + 1) * DK, :])

        def v_unit(ni, half):
            def f():
                lo, hi = (0, 512) if half == 0 else (512, TOTAL)
                ps = proj_ps.tile([128, 512], F32, tag="proj", name="proj_v")
                for ec in range(EC):
                    xs = (xtea_big[:, ec, ni * 128:(ni + 1) * 128] if ni < 4
                          else xteb_big[:, ec, (ni - 4) * 128:(ni - 3) * 128])
                    nc.tensor.matmul(ps[:, 0:hi - lo], lhsT=xs,
                                     rhs=wv_big[:, ec, lo:hi], start=(ec == 0), stop=(ec == EC - 1))
                hlo, hhi = lo // DK, hi // DK
                nc.vector.tensor_copy(
                    vsb[ni][:, hlo:hhi, 0:DK],
                    ps[:, 0:hi - lo].rearrange("p (h d) -> p h d", d=DK))
                if half == 1:
                    nc.vector.memset(vsb[ni][:, :, DK], 1.0)
            return (1.3, f)

        def av_unit(h, attns, g):
            def f():
                emit_av(h, attns, g)
            return (0.5, f)

        def pump(budget):
            while units and budget > 0:
                c, f = units.popleft()
                f()
                budget -= c

        # minimal head-0 critical path up front: kt half0 + qt q0/q1 only
        emit_kt_half(0, 0)
        pieces[0].add("k0")
        qt_unit(0, 0)[1]()
        qt_unit(0, 1)[1]()
        units.append(kt_unit(0, 1))
        units.append(qt_unit(0, 2))
        units.append(qt_unit(0, 3))
        for ni in range(NI):
            units.append(v_unit(ni, 0))
            units.append(v_unit(ni, 1))

        def need(p, ni, half):
            req = {"k0" if ni < 4 else "k1", f"q{2 * half}", f"q{2 * half + 1}"}
            while not req <= pieces.get(p, set()):
                c, f = units.popleft()
                f()

        slot = 0
        for h in range(H):
            p = h // 2
            if h % 2 == 1 and p + 1 <= PAIRS - 1:
                # next pair's projections jump the queue so the even-head
                # boundary never has to force-drain a big batch
                for mh in range(3, -1, -1):
                    units.appendleft(qt_unit(p + 1, mh))
                units.appendleft(kt_unit(p + 1, 1))
                units.appendleft(kt_unit(p + 1, 0))
            attns = [attn_pool.tile([128, M], BF16, tag="attn", name=f"attn_t{h}_{ni}")
                     for ni in range(NI)]
            for half in range(2):
                for ni in range(NI):
                    need(p, ni, half)
                    emit_scores_exp_half(h, qts[h], ni, half, attns[ni])
                    pump(2.5 if slot < 12 else 0.75)
                    slot += 1
            qts[h] = None  # allow qpt slot reuse
            for g in range(8):
                units.append(av_unit(h, attns, g))
        while units:
            c, f = units.popleft()
            f()

    nc.compile()
    return nc


def _get_nc():
    if "nc" not in _CACHE:
        _CACHE["nc"] = _build()
    return _CACHE["nc"]


def kernel(**inputs):
    global LAST_EXEC_NS, LAST_TRACE_DIR
    from concourse.bass_utils import run_bass_kernel_spmd

    ehr = np.asarray(inputs["ehr_embeddings"], dtype=np.float32)
    mi = np.asarray(inputs["missing_indices"]).astype(np.int64)
    ei = np.asarray(inputs["exist_indices"]).astype(np.int64)
    Wq = np.asarray(inputs["Wq"], dtype=np.float32)
    Wk = np.asarray(inputs["Wk"], dtype=np.float32)
    Wv = np.asarray(inputs["Wv"], dtype=np.float32)
    bq = np.asarray(inputs["bq"], dtype=np.float32)
    bv = np.asarray(inputs["bv"], dtype=np.float32)
    cooc = np.asarray(inputs["cooc_bias"], dtype=np.float32)
    # bk is softmax-shift-invariant (adds a per-query constant to scores);
    # dropped on device, consistent across cores so the combine is exact.

    scale = 1.0 / np.sqrt(np.float32(DK))
    bq_s = np.ascontiguousarray((bq * scale).reshape(PAIRS, 128).T)

    def fold(a):  # [E, F] -> [128, EC, F] (partition-major chunk fold)
        return a.reshape(EC, 128, a.shape[1]).transpose(1, 0, 2)

    def wfold(a):  # [E, TOTAL] -> [128, PAIRS, EC, 128] (pair-col major)
        return np.ascontiguousarray(
            fold(a).reshape(128, EC, PAIRS, 128).transpose(0, 2, 1, 3))

    wq_s = wfold((Wq * scale).astype(np.float16))
    wk_s = wfold(Wk.astype(np.float16))
    wv_s = np.ascontiguousarray(fold(Wv.astype(np.float16)))
    missing_emb = ehr[mi]                       # [M, E]
    xt_m = np.ascontiguousarray(
        fold(missing_emb.T.astype(np.float16))
        .reshape(128, EC, 4, 512).transpose(0, 2, 1, 3))  # [128, 4, EC, 512]
    mbt = np.ascontiguousarray(
        cooc[:, mi, :].transpose(0, 2, 1).reshape(H * DK, M).astype(np.float16))

    common = {"xt_m": xt_m, "mbt": mbt, "wq": wq_s, "wk": wk_s, "wv": wv_s,
              "bq": bq_s}
    in_maps = []
    for c in range(CORES):
        eic = ei[c * NLOC:(c + 1) * NLOC]
        xte_f = fold(ehr[eic].T.astype(np.float16))  # [128, EC, NLOC]
        ebt = np.ascontiguousarray(
            cooc[:, eic, :].transpose(0, 2, 1).reshape(H * DK, NLOC).astype(np.float16))
        in_maps.append({**common, "xt_ea": np.ascontiguousarray(xte_f[:, :, 0:512]),
                        "xt_eb": np.ascontiguousarray(xte_f[:, :, 512:NLOC]),
                        "ebt": ebt})

    nc = _get_nc()
    trace = os.environ.get("KERNEL_TRACE") == "1"
    kwargs = {}
    if trace:
        import tempfile
        LAST_TRACE_DIR = tempfile.mkdtemp(prefix="kern_trace_")
        kwargs = {"trace": True, "tmpdir": LAST_TRACE_DIR}
        try:
            import ntff_shim
            ntff_shim.install()
        except ImportError:
            pass
    res = run_bass_kernel_spmd(nc, in_maps, list(range(CORES)), **kwargs)
    LAST_EXEC_NS = res.exec_time_ns

    # ---- host combine ----
    num = np.zeros((H, M, DK), dtype=np.float64)
    den = np.zeros((H, M), dtype=np.float64)
    for c in range(CORES):
        op = res.results[c]["out_p"].astype(np.float64)  # [H, M, DK+1]
        num += op[:, :, :DK]
        den += op[:, :, DK]
    out = num / den[:, :, None]                          # [H, M, DK]
    out = out.transpose(1, 0, 2).reshape(M, TOTAL) + bv.astype(np.float64)
    result = ehr.copy()
    result[mi] = out.astype(np.float32)
    return result


# revision 20
# speedup vs baseline: 1.1052x; 1.0329x over previous
"""MultiHeadSectionAttentionImputer on 8 TRN2 NeuronCores (Bass/Tile).

Sharding: 2 head-groups x 4 key-shards. Core c handles heads
[6*(c//4), 6*(c//4)+6) and exist-keys [1536*(c%4), 1536*(c%4)+1536).
Each core:
  - projects its key shard to K,V (K = X_e @ Wk; V = X_e @ Wv with an
    appended ones column), its 6 heads only
  - projects the full missing set to Q for its 6 heads (Wq,bq pre-scaled
    by 1/sqrt(d_k) on host; bk dropped - it only shifts scores by a
    per-query constant, softmax-invariant and consistent across shards)
  - computes scoresT[key, query] per head with a fused 128-deep
    contraction: d' = [q-dims(64) | cooc-bias-dims(64)] so one matmul
    yields q.k/sqrt(dk) + mb.eb
  - exp() without max subtraction (scores bounded ~<60, safe in fp32)
  - attn @ [V | 1] accumulated over the 12 key chunks -> partial
    numerators (64 cols) + denominator per query
Host combines partials across the 4 key-shards of each head group
(exact softmax over all 6144 keys), adds bv, scatters into ehr.

Matmul inputs are fp16 (psum accumulates fp32); the attention weights
are bf16 (exp output needs fp32-like range; no max subtraction).
"""

import os
import sys
import numpy as np
from contextlib import ExitStack

sys.path.insert(0, "/opt/trn_rl_repo")

# problem constants (hardcoded; kernel.py must be self-contained)
H = 12          # total heads
DK = 64         # head dim
E = 768         # embed dim
TOTAL = H * DK  # 768
M = 2048        # missing sections
N = 6144        # existing sections
CORES = 8
HGROUPS = 2     # head groups (cores 0-3 -> heads 0-5, cores 4-7 -> 6-11)
NSHARDS = 4
HH = H // HGROUPS        # 6 heads per core
PP = HH // 2             # 3 head pairs per core
TT = HH * DK             # 384 projection cols per core
NLOC = N // NSHARDS      # 1536 keys per core
EC = E // 128            # 6 contraction chunks
NI = NLOC // 128         # 12 key chunks per core
MI = M // 128            # 16 query chunks

_CACHE = {}
LAST_EXEC_NS = None
LAST_TRACE_DIR = None


def _build():
    import concourse.bass as bass
    import concourse.tile as tile
    from concourse import bacc, mybir
    from collections import deque

    F32 = mybir.dt.float32
    FP16 = mybir.dt.float16
    BF16 = mybir.dt.bfloat16
    Exp = mybir.ActivationFunctionType.Exp

    nc = bacc.Bacc("TRN2", target_bir_lowering=False, debug=False)

    # ---- I/O (layouts chosen so every DMA is contiguous) ----
    xt_m = nc.dram_tensor("xt_m", [128, 4, EC, 512], FP16, kind="ExternalInput").ap()
    mbt = nc.dram_tensor("mbt", [HH * DK, M], FP16, kind="ExternalInput").ap()
    xt_e = nc.dram_tensor("xt_e", [128, 3, EC, 512], FP16, kind="ExternalInput").ap()
    ebt = nc.dram_tensor("ebt", [HH * DK, NLOC], FP16, kind="ExternalInput").ap()
    wq = nc.dram_tensor("wq", [128, PP, EC, 128], FP16, kind="ExternalInput").ap()
    wk = nc.dram_tensor("wk", [128, PP, EC, 128], FP16, kind="ExternalInput").ap()
    wv = nc.dram_tensor("wv", [128, EC, TT], FP16, kind="ExternalInput").ap()
    bq = nc.dram_tensor("bq", [128, PP], F32, kind="ExternalInput").ap()
    out_p = nc.dram_tensor("out_p", [HH, M, DK + 1], F32, kind="ExternalOutput").ap()

    with tile.TileContext(nc) as tc, ExitStack() as ctx:
        persist = ctx.enter_context(tc.tile_pool(name="persist", bufs=1))
        qpt_pool = ctx.enter_context(tc.tile_pool(name="qpt", bufs=4))
        attn_pool = ctx.enter_context(tc.tile_pool(name="attn", bufs=20))
        osb_pool = ctx.enter_context(tc.tile_pool(name="osb", bufs=12))
        proj_ps = ctx.enter_context(tc.tile_pool(name="proj_ps", bufs=1, space="PSUM"))
        sc_ps = ctx.enter_context(tc.tile_pool(name="sc_ps", bufs=2, space="PSUM"))
        av_ps = ctx.enter_context(tc.tile_pool(name="av_ps", bufs=3, space="PSUM"))

        # K'T per head [128, NLOC]: rows = k-dims | eb-dims (parity layout:
        # even head k at partitions 0:64, odd head k at 64:128 - avoids any
        # cross-partition copies; scores only need a consistent d' order)
        kpt = [persist.tile([128, NLOC], FP16, tag=f"kpt{h}", name=f"kpt{h}")
               for h in range(HH)]
        # V per key chunk [128, HH, DK+1] bf16, ones col at [., ., DK]
        vsb = [persist.tile([128, HH, DK + 1], BF16, tag=f"v{ni}", name=f"v{ni}")
               for ni in range(NI)]
        bq_sb = persist.tile([128, PP], F32, tag="bq")
        wk_big = persist.tile([128, PP, EC, 128], FP16, tag="wk")
        wq_big = persist.tile([128, PP, EC, 128], FP16, tag="wq")
        wv_big = persist.tile([128, EC, TT], FP16, tag="wv")
        xte_big = persist.tile([128, 3, EC, 512], FP16, tag="xte")
        xtm_big = persist.tile([128, 4, EC, 512], FP16, tag="xtm")
        q0_0 = qpt_pool.tile([128, M], FP16, tag="qpt", name="qpt0")
        q1_0 = qpt_pool.tile([128, M], FP16, tag="qpt", name="qpt1")

        # input DMAs: criticals (pair-0 / first key-third) on dedicated
        # queues, bulk remainder behind them
        nc.scalar.dma_start(bq_sb[:], bq)
        nc.sync.dma_start(wk_big[:, 0], wk[:, 0])
        nc.gpsimd.dma_start(q0_0[64:128, :], mbt[0:DK, :])
        nc.gpsimd.dma_start(q1_0[0:64, :], mbt[DK:2 * DK, :])
        nc.gpsimd.dma_start(kpt[0][64:128, :], ebt[0:DK, :])
        nc.gpsimd.dma_start(kpt[1][0:64, :], ebt[DK:2 * DK, :])
        nc.scalar.dma_start(xte_big[:, 0], xt_e[:, 0])
        nc.gpsimd.dma_start(wq_big[:, 0], wq[:, 0])
        nc.sync.dma_start(xte_big[:, 1], xt_e[:, 1])
        nc.scalar.dma_start(xtm_big[:, 0], xt_m[:, 0])
        nc.gpsimd.dma_start(wv_big[:], wv[:])
        nc.sync.dma_start(xte_big[:, 2], xt_e[:, 2])
        nc.scalar.dma_start(xtm_big[:, 1:4], xt_m[:, 1:4])
        nc.gpsimd.dma_start(wk_big[:, 1:PP], wk[:, 1:PP])
        nc.gpsimd.dma_start(wq_big[:, 1:PP], wq[:, 1:PP])

        def emit_scores_exp_half(h, qt, ni, half, at):
            """scoresT half [128 keys, 1024 queries] + exp into attnT.
            [128,1024] scores psum tiles double-buffer (2 banks each) so
            ACT runs exp back-to-back with no psum-free wait."""
            ps = sc_ps.tile([128, 1024], F32, tag="sc", name="sc_ps_t")
            mo = half * 1024
            for mj in range(2):
                nc.tensor.matmul(
                    ps[:, mj * 512:(mj + 1) * 512],
                    lhsT=kpt[h][:, ni * 128:(ni + 1) * 128],
                    rhs=qt[:, mo + mj * 512:mo + (mj + 1) * 512],
                    start=True, stop=True)
            nc.scalar.activation(at[:, mo:mo + 1024], ps[:], Exp)

        def emit_av(h, attns, g):
            """out chunks [128 queries, DK+1], mi in [2g, 2g+2); the bf16
            attnT chunk is the stationary operand (fast weight load)."""
            for mi in range(2 * g, 2 * g + 2):
                ps = av_ps.tile([128, DK + 1], F32, tag="av", name="av_ps_t")
                for ni in range(NI):
                    nc.tensor.matmul(
                        ps[:], lhsT=attns[ni][:, mi * 128:(mi + 1) * 128],
                        rhs=vsb[ni][:, h, :],
                        start=(ni == 0), stop=(ni == NI - 1))
                ot = osb_pool.tile([128, DK + 1], F32, tag="osb", name="osb_t")
                nc.vector.tensor_copy(ot[:], ps[:])
                nc.sync.dma_start(out_p[h, mi * 128:(mi + 1) * 128, :], ot[:])

        # ---- unit-queue scheduler ----
        units = deque()
        qts = {0: q0_0, 1: q1_0}
        pieces = {0: set()}  # pair -> done piece ids (k0..k2, q0..q3)

        def emit_qt_quarter(p, mh, q0, q1):
            ps = proj_ps.tile([128, 512], F32, tag="proj", name="proj_qt")
            for ec in range(EC):
                nc.tensor.matmul(ps[:], lhsT=wq_big[:, p, ec, :],
                                 rhs=xtm_big[:, mh, ec, :],
                                 start=(ec == 0), stop=(ec == EC - 1))
            mo = mh * 512
            nc.vector.tensor_scalar_add(
                q0[0:64, mo:mo + 512], ps[0:64, :], bq_sb[0:64, p:p + 1])
            nc.vector.tensor_scalar_add(
                q1[64:128, mo:mo + 512], ps[64:128, :], bq_sb[64:128, p:p + 1])

        def qt_unit(p, mh):
            def f():
                pieces.setdefault(p, set()).add(f"q{mh}")
                q0, q1 = qts.get(2 * p), qts.get(2 * p + 1)
                if q0 is None:
                    q0 = qpt_pool.tile([128, M], FP16, tag="qpt", name=f"qpt{2*p}")
                    q1 = qpt_pool.tile([128, M], FP16, tag="qpt", name=f"qpt{2*p+1}")
                    h0, h1 = 2 * p, 2 * p + 1
                    nc.sync.dma_start(q0[64:128, :], mbt[h0 * DK:(h0 + 1) * DK, :])
                    nc.sync.dma_start(q1[0:64, :], mbt[h1 * DK:(h1 + 1) * DK, :])
                    qts[2 * p], qts[2 * p + 1] = q0, q1
                emit_qt_quarter(p, mh, q0, q1)
            return (1.35, f)

        def emit_kt_third(p, t):
            h0, h1 = 2 * p, 2 * p + 1
            lo = t * 512
            ps = proj_ps.tile([128, 512], F32, tag="proj", name="proj_kt")
            for ec in range(EC):
                nc.tensor.matmul(ps[:], lhsT=wk_big[:, p, ec, :],
                                 rhs=xte_big[:, t, ec, :],
                                 start=(ec == 0), stop=(ec == EC - 1))
            nc.vector.tensor_copy(kpt[h0][0:64, lo:lo + 512], ps[0:64, :])
            nc.vector.tensor_copy(kpt[h1][64:128, lo:lo + 512], ps[64:128, :])
            if t == 0 and p > 0:
                nc.sync.dma_start(kpt[h0][64:128, :], ebt[h0 * DK:(h0 + 1) * DK, :])
                nc.sync.dma_start(kpt[h1][0:64, :], ebt[h1 * DK:(h1 + 1) * DK, :])

        def kt_unit(p, t):
            def f():
                pieces.setdefault(p, set()).add(f"k{t}")
                emit_kt_third(p, t)
            return (1.35, f)

        def v_unit(ni):
            def f():
                ps = proj_ps.tile([128, TT], F32, tag="proj", name="proj_v")
                t, off = divmod(ni, 4)
                for ec in range(EC):
                    nc.tensor.matmul(
                        ps[:], lhsT=xte_big[:, t, ec, off * 128:(off + 1) * 128],
                        rhs=wv_big[:, ec, :], start=(ec == 0), stop=(ec == EC - 1))
                nc.vector.tensor_copy(
                    vsb[ni][:, :, 0:DK], ps[:].rearrange("p (h d) -> p h d", d=DK))
                nc.vector.memset(vsb[ni][:, :, DK], 1.0)
            return (1.0, f)

        def av_unit(h, attns, g):
            def f():
                emit_av(h, attns, g)
            return (0.8, f)

        def pump(budget):
            while units and budget > 0:
                c, f = units.popleft()
                f()
                budget -= c

        # minimal head-0 critical path up front: kt third0 + qt q0/q1
        emit_kt_third(0, 0)
        pieces[0].add("k0")
        qt_unit(0, 0)[1]()
        qt_unit(0, 1)[1]()
        units.append(kt_unit(0, 1))
        units.append(kt_unit(0, 2))
        units.append(qt_unit(0, 2))
        units.append(qt_unit(0, 3))
        for ni in range(NI):
            units.append(v_unit(ni))

        def need(p, ni, half):
            req = {f"k{ni // 4}", f"q{2 * half}", f"q{2 * half + 1}"}
            while not req <= pieces.get(p, set()):
                c, f = units.popleft()
                f()

        slot = 0
        for h in range(HH):
            p = h // 2
            if h % 2 == 1 and p + 1 <= PP - 1:
                # next pair's projections jump the queue (front) so the
                # even-head boundary never force-drains a big batch
                for mh in range(3, -1, -1):
                    units.appendleft(qt_unit(p + 1, mh))
                for t in range(2, -1, -1):
                    units.appendleft(kt_unit(p + 1, t))
            attns = [attn_pool.tile([128, M], BF16, tag="attn",
                                    name=f"attn_{h}_{ni}") for ni in range(NI)]
            for half in range(2):
                for ni in range(NI):
                    need(p, ni, half)
                    emit_scores_exp_half(h, qts[h], ni, half, attns[ni])
                    pump(2.0 if slot < 16 else 0.7)
                    slot += 1
            qts[h] = None  # release the qpt slot
            for g in range(8):
                units.append(av_unit(h, attns, g))
        while units:
            c, f = units.popleft()
            f()

    nc.compile()
    return nc


def _get_nc():
    if "nc" not in _CACHE:
        _CACHE["nc"] = _build()
    return _CACHE["nc"]


def kernel(**inputs):
    global LAST_EXEC_NS, LAST_TRACE_DIR
    from concourse.bass_utils import run_bass_kernel_spmd

    ehr = np.asarray(inputs["ehr_embeddings"], dtype=np.float32)
    mi = np.asarray(inputs["missing_indices"]).astype(np.int64)
    ei = np.asarray(inputs["exist_indices"]).astype(np.int64)
    Wq = np.asarray(inputs["Wq"], dtype=np.float32)
    Wk = np.asarray(inputs["Wk"], dtype=np.float32)
    Wv = np.asarray(inputs["Wv"], dtype=np.float32)
    bq = np.asarray(inputs["bq"], dtype=np.float32)
    bv = np.asarray(inputs["bv"], dtype=np.float32)
    cooc = np.asarray(inputs["cooc_bias"], dtype=np.float32)

    scale = 1.0 / np.sqrt(np.float32(DK))

    def fold(a):  # [E, F] -> [128, EC, F]
        return a.reshape(EC, 128, a.shape[1]).transpose(1, 0, 2)

    def wfold(a):  # [E, TT] -> [128, PP, EC, 128] (pair-col major)
        return np.ascontiguousarray(
            fold(a).reshape(128, EC, PP, 128).transpose(0, 2, 1, 3))

    missing_emb = ehr[mi]                       # [M, E]
    xt_m = np.ascontiguousarray(
        fold(missing_emb.T.astype(np.float16))
        .reshape(128, EC, 4, 512).transpose(0, 2, 1, 3))  # [128, 4, EC, 512]
    wq_all = (Wq * scale).astype(np.float16)
    wk_all = Wk.astype(np.float16)
    wv_all = Wv.astype(np.float16)
    mbt_all = cooc[:, mi, :].transpose(0, 2, 1).reshape(H * DK, M).astype(np.float16)
    bq_all = (bq * scale).astype(np.float32)

    in_maps = []
    for c in range(CORES):
        hg, ns = c // NSHARDS, c % NSHARDS
        hsl = slice(hg * TT, (hg + 1) * TT)
        eic = ei[ns * NLOC:(ns + 1) * NLOC]
        xte_f = fold(ehr[eic].T.astype(np.float16))  # [128, EC, NLOC]
        xt_e = np.ascontiguousarray(
            xte_f.reshape(128, EC, 3, 512).transpose(0, 2, 1, 3))
        ebt = np.ascontiguousarray(
            cooc[hg * HH:(hg + 1) * HH, eic, :].transpose(0, 2, 1)
            .reshape(HH * DK, NLOC).astype(np.float16))
        in_maps.append({
            "xt_m": xt_m,
            "mbt": np.ascontiguousarray(mbt_all[hsl]),
            "xt_e": xt_e, "ebt": ebt,
            "wq": wfold(wq_all[:, hsl]),
            "wk": wfold(wk_all[:, hsl]),
            "wv": np.ascontiguousarray(fold(wv_all[:, hsl])),
            "bq": np.ascontiguousarray(bq_all[hsl].reshape(PP, 128).T),
        })

    nc = _get_nc()
    kwargs = {}
    if os.environ.get("KERNEL_TRACE") == "1":
        import tempfile
        LAST_TRACE_DIR = tempfile.mkdtemp(prefix="kern_trace_")
        kwargs = {"trace": True, "tmpdir": LAST_TRACE_DIR}
        try:
            import ntff_shim
            ntff_shim.install()
        except ImportError:
            pass
    res = run_bass_kernel_spmd(nc, in_maps, list(range(CORES)), **kwargs)
    LAST_EXEC_NS = res.exec_time_ns

    # ---- host combine (exact softmax across the 4 key shards) ----
    num = np.zeros((H, M, DK), dtype=np.float64)
    den = np.zeros((H, M), dtype=np.float64)
    for c in range(CORES):
        hg = c // NSHARDS
        op = res.results[c]["out_p"].astype(np.float64)  # [HH, M, DK+1]
        num[hg * HH:(hg + 1) * HH] += op[:, :, :DK]
        den[hg * HH:(hg + 1) * HH] += op[:, :, DK]
    out = num / den[:, :, None]                          # [H, M, DK]
    out = out.transpose(1, 0, 2).reshape(M, TOTAL) + bv.astype(np.float64)
    result = ehr.copy()
    result[mi] = out.astype(np.float32)
    return result


# revision 21
# speedup vs baseline: 1.1432x; 1.0344x over previous
"""MultiHeadSectionAttentionImputer on 8 TRN2 NeuronCores (Bass/Tile).

Sharding: 2 head-groups x 4 key-shards. Core c handles heads
[6*(c//4), 6*(c//4)+6) and exist-keys [1536*(c%4), 1536*(c%4)+1536).
Each core:
  - projects its key shard to K,V (K = X_e @ Wk; V = X_e @ Wv with an
    appended ones column), its 6 heads only
  - projects the full missing set to Q for its 6 heads (Wq,bq pre-scaled
    by 1/sqrt(d_k) on host; bk dropped - it only shifts scores by a
    per-query constant, softmax-invariant and consistent across shards)
  - computes scoresT[key, query] per head with a fused 128-deep
    contraction: d' = [q-dims(64) | cooc-bias-dims(64)] so one matmul
    yields q.k/sqrt(dk) + mb.eb
  - exp() without max subtraction (scores bounded ~<60, safe in fp32)
  - attn @ [V | 1] accumulated over the 12 key chunks -> partial
    numerators (64 cols) + denominator per query
Host combines partials across the 4 key-shards of each head group
(exact softmax over all 6144 keys), adds bv, scatters into ehr.

Matmul inputs are fp16 (psum accumulates fp32); the attention weights
are bf16 (exp output needs fp32-like range; no max subtraction).
"""

import os
import sys
import numpy as np
from contextlib import ExitStack

sys.path.insert(0, "/opt/trn_rl_repo")

# problem constants (hardcoded; kernel.py must be self-contained)
H = 12          # total heads
DK = 64         # head dim
E = 768         # embed dim
TOTAL = H * DK  # 768
M = 2048        # missing sections
N = 6144        # existing sections
CORES = 8
HGROUPS = 2     # head groups (cores 0-3 -> heads 0-5, cores 4-7 -> 6-11)
NSHARDS = 4
HH = H // HGROUPS        # 6 heads per core
PP = HH // 2             # 3 head pairs per core
TT = HH * DK             # 384 projection cols per core
NLOC = N // NSHARDS      # 1536 keys per core
EC = E // 128            # 6 contraction chunks
NI = NLOC // 128         # 12 key chunks per core
MI = M // 128            # 16 query chunks

_CACHE = {}
LAST_EXEC_NS = None
LAST_TRACE_DIR = None


def _build():
    import concourse.bass as bass
    import concourse.tile as tile
    from concourse import bacc, mybir
    from collections import deque

    F32 = mybir.dt.float32
    FP16 = mybir.dt.float16
    BF16 = mybir.dt.bfloat16
    Exp = mybir.ActivationFunctionType.Exp

    nc = bacc.Bacc("TRN2", target_bir_lowering=False, debug=False)

    # ---- I/O (layouts chosen so every DMA is contiguous) ----
    xt_m = nc.dram_tensor("xt_m", [128, 4, EC, 512], FP16, kind="ExternalInput").ap()
    mbt = nc.dram_tensor("mbt", [HH * DK, M], FP16, kind="ExternalInput").ap()
    xt_e = nc.dram_tensor("xt_e", [128, 3, EC, 512], FP16, kind="ExternalInput").ap()
    ebt = nc.dram_tensor("ebt", [HH * DK, NLOC], FP16, kind="ExternalInput").ap()
    wq = nc.dram_tensor("wq", [128, PP, EC, 128], FP16, kind="ExternalInput").ap()
    wk = nc.dram_tensor("wk", [128, PP, EC, 128], FP16, kind="ExternalInput").ap()
    wv = nc.dram_tensor("wv", [128, EC, TT], FP16, kind="ExternalInput").ap()
    bq = nc.dram_tensor("bq", [128, PP], F32, kind="ExternalInput").ap()
    out_p = nc.dram_tensor("out_p", [HH, M, DK + 1], F32, kind="ExternalOutput").ap()

    with tile.TileContext(nc) as tc, ExitStack() as ctx:
        persist = ctx.enter_context(tc.tile_pool(name="persist", bufs=1))
        qpt_pool = ctx.enter_context(tc.tile_pool(name="qpt", bufs=4))
        attn_pool = ctx.enter_context(tc.tile_pool(name="attn", bufs=20))
        osb_pool = ctx.enter_context(tc.tile_pool(name="osb", bufs=12))
        proj_ps = ctx.enter_context(tc.tile_pool(name="proj_ps", bufs=2, space="PSUM"))
        sc_ps = ctx.enter_context(tc.tile_pool(name="sc_ps", bufs=2, space="PSUM"))
        av_ps = ctx.enter_context(tc.tile_pool(name="av_ps", bufs=2, space="PSUM"))

        # K'T per head [128, NLOC]: rows = k-dims | eb-dims (parity layout:
        # even head k at partitions 0:64, odd head k at 64:128 - avoids any
        # cross-partition copies; scores only need a consistent d' order)
        kpt = [persist.tile([128, NLOC], FP16, tag=f"kpt{h}", name=f"kpt{h}")
               for h in range(HH)]
        # V per key chunk [128, HH, DK+1] bf16, ones col at [., ., DK]
        vsb = [persist.tile([128, HH, DK + 1], BF16, tag=f"v{ni}", name=f"v{ni}")
               for ni in range(NI)]
        bq_sb = persist.tile([128, PP], F32, tag="bq")
        wk_big = persist.tile([128, PP, EC, 128], FP16, tag="wk")
        wq_big = persist.tile([128, PP, EC, 128], FP16, tag="wq")
        wv_big = persist.tile([128, EC, TT], FP16, tag="wv")
        xte_big = persist.tile([128, 3, EC, 512], FP16, tag="xte")
        xtm_big = persist.tile([128, 4, EC, 512], FP16, tag="xtm")
        q0_0 = qpt_pool.tile([128, M], FP16, tag="qpt", name="qpt0")
        q1_0 = qpt_pool.tile([128, M], FP16, tag="qpt", name="qpt1")

        # input DMAs: criticals (pair-0 / first key-third) on dedicated
        # queues, bulk remainder behind them
        nc.scalar.dma_start(bq_sb[:], bq)
        nc.sync.dma_start(wk_big[:, 0], wk[:, 0])
        nc.gpsimd.dma_start(q0_0[64:128, :], mbt[0:DK, :])
        nc.gpsimd.dma_start(q1_0[0:64, :], mbt[DK:2 * DK, :])
        nc.gpsimd.dma_start(kpt[0][64:128, :], ebt[0:DK, :])
        nc.gpsimd.dma_start(kpt[1][0:64, :], ebt[DK:2 * DK, :])
        nc.scalar.dma_start(xte_big[:, 0], xt_e[:, 0])
        nc.gpsimd.dma_start(wq_big[:, 0], wq[:, 0])
        nc.sync.dma_start(xte_big[:, 1], xt_e[:, 1])
        nc.scalar.dma_start(xtm_big[:, 0], xt_m[:, 0])
        nc.gpsimd.dma_start(wv_big[:], wv[:])
        nc.sync.dma_start(xte_big[:, 2], xt_e[:, 2])
        nc.scalar.dma_start(xtm_big[:, 1:4], xt_m[:, 1:4])
        nc.gpsimd.dma_start(wk_big[:, 1:PP], wk[:, 1:PP])
        nc.gpsimd.dma_start(wq_big[:, 1:PP], wq[:, 1:PP])

        def emit_scores_exp_half(h, qt, ni, half, at):
            """scoresT half [128 keys, 1024 queries] + exp into attnT.
            [128,1024] scores psum tiles double-buffer (2 banks each) so
            ACT runs exp back-to-back with no psum-free wait."""
            ps = sc_ps.tile([128, 1024], F32, tag="sc", name="sc_ps_t")
            mo = half * 1024
            for mj in range(2):
                nc.tensor.matmul(
                    ps[:, mj * 512:(mj + 1) * 512],
                    lhsT=kpt[h][:, ni * 128:(ni + 1) * 128],
                    rhs=qt[:, mo + mj * 512:mo + (mj + 1) * 512],
                    start=True, stop=True)
            nc.scalar.activation(at[:, mo:mo + 1024], ps[:], Exp)

        def emit_av(h, attns, g):
            """out chunks [128 queries, DK+1], mi in [2g, 2g+2); the bf16
            attnT chunk is the stationary operand (fast weight load)."""
            for mi in range(2 * g, 2 * g + 2):
                ps = av_ps.tile([128, DK + 1], F32, tag="av", name="av_ps_t")
                for ni in range(NI):
                    nc.tensor.matmul(
                        ps[:], lhsT=attns[ni][:, mi * 128:(mi + 1) * 128],
                        rhs=vsb[ni][:, h, :],
                        start=(ni == 0), stop=(ni == NI - 1))
                ot = osb_pool.tile([128, DK + 1], F32, tag="osb", name="osb_t")
                nc.vector.tensor_copy(ot[:], ps[:])
                nc.sync.dma_start(out_p[h, mi * 128:(mi + 1) * 128, :], ot[:])

        # ---- unit-queue scheduler ----
        units = deque()
        qts = {0: q0_0, 1: q1_0}
        pieces = {0: set()}  # pair -> done piece ids (k0..k2, q0..q3)

        def emit_qt_quarter(p, mh, q0, q1):
            ps = proj_ps.tile([128, 512], F32, tag="proj", name="proj_qt")
            for ec in range(EC):
                nc.tensor.matmul(ps[:], lhsT=wq_big[:, p, ec, :],
                                 rhs=xtm_big[:, mh, ec, :],
                                 start=(ec == 0), stop=(ec == EC - 1))
            mo = mh * 512
            nc.vector.tensor_scalar_add(
                q0[0:64, mo:mo + 512], ps[0:64, :], bq_sb[0:64, p:p + 1])
            nc.vector.tensor_scalar_add(
                q1[64:128, mo:mo + 512], ps[64:128, :], bq_sb[64:128, p:p + 1])

        def qt_unit(p, mh):
            def f():
                pieces.setdefault(p, set()).add(f"q{mh}")
                q0, q1 = qts.get(2 * p), qts.get(2 * p + 1)
                if q0 is None:
                    q0 = qpt_pool.tile([128, M], FP16, tag="qpt", name=f"qpt{2*p}")
                    q1 = qpt_pool.tile([128, M], FP16, tag="qpt", name=f"qpt{2*p+1}")
                    h0, h1 = 2 * p, 2 * p + 1
                    nc.sync.dma_start(q0[64:128, :], mbt[h0 * DK:(h0 + 1) * DK, :])
                    nc.sync.dma_start(q1[0:64, :], mbt[h1 * DK:(h1 + 1) * DK, :])
                    qts[2 * p], qts[2 * p + 1] = q0, q1
                emit_qt_quarter(p, mh, q0, q1)
            return (1.35, f)

        def emit_kt_third(p, t):
            h0, h1 = 2 * p, 2 * p + 1
            lo = t * 512
            ps = proj_ps.tile([128, 512], F32, tag="proj", name="proj_kt")
            for ec in range(EC):
                nc.tensor.matmul(ps[:], lhsT=wk_big[:, p, ec, :],
                                 rhs=xte_big[:, t, ec, :],
                                 start=(ec == 0), stop=(ec == EC - 1))
            nc.vector.tensor_copy(kpt[h0][0:64, lo:lo + 512], ps[0:64, :])
            nc.vector.tensor_copy(kpt[h1][64:128, lo:lo + 512], ps[64:128, :])
            if t == 0 and p > 0:
                nc.sync.dma_start(kpt[h0][64:128, :], ebt[h0 * DK:(h0 + 1) * DK, :])
                nc.sync.dma_start(kpt[h1][0:64, :], ebt[h1 * DK:(h1 + 1) * DK, :])

        def kt_unit(p, t):
            def f():
                pieces.setdefault(p, set()).add(f"k{t}")
                emit_kt_third(p, t)
            return (1.35, f)

        def v_unit(ni):
            def f():
                ps = proj_ps.tile([128, TT], F32, tag="proj", name="proj_v")
                t, off = divmod(ni, 4)
                for ec in range(EC):
                    nc.tensor.matmul(
                        ps[:], lhsT=xte_big[:, t, ec, off * 128:(off + 1) * 128],
                        rhs=wv_big[:, ec, :], start=(ec == 0), stop=(ec == EC - 1))
                nc.vector.tensor_copy(
                    vsb[ni][:, :, 0:DK], ps[:].rearrange("p (h d) -> p h d", d=DK))
                nc.vector.memset(vsb[ni][:, :, DK], 1.0)
            return (1.0, f)

        def av_unit(h, attns, g):
            def f():
                emit_av(h, attns, g)
            return (0.8, f)

        def pump(budget):
            while units and budget > 0:
                c, f = units.popleft()
                f()
                budget -= c

        # minimal head-0 critical path up front: kt third0 + qt q0/q1
        emit_kt_third(0, 0)
        pieces[0].add("k0")
        qt_unit(0, 0)[1]()
        qt_unit(0, 1)[1]()
        units.append(kt_unit(0, 1))
        units.append(kt_unit(0, 2))
        units.append(qt_unit(0, 2))
        units.append(qt_unit(0, 3))
        for ni in range(NI):
            units.append(v_unit(ni))

        def need(p, ni, half):
            req = {f"k{ni // 4}", f"q{2 * half}", f"q{2 * half + 1}"}
            while not req <= pieces.get(p, set()):
                c, f = units.popleft()
                f()

        slot = 0
        for h in range(HH):
            p = h // 2
            if h % 2 == 1 and p + 1 <= PP - 1:
                # next pair's projections jump the queue (front) so the
                # even-head boundary never force-drains a big batch
                for mh in range(3, -1, -1):
                    units.appendleft(qt_unit(p + 1, mh))
                for t in range(2, -1, -1):
                    units.appendleft(kt_unit(p + 1, t))
            attns = [attn_pool.tile([128, M], BF16, tag="attn",
                                    name=f"attn_{h}_{ni}") for ni in range(NI)]
            for half in range(2):
                for ni in range(NI):
                    need(p, ni, half)
                    emit_scores_exp_half(h, qts[h], ni, half, attns[ni])
                    pump(2.0 if slot < 16 else 0.7)
                    slot += 1
            qts[h] = None  # release the qpt slot
            for g in range(8):
                units.append(av_unit(h, attns, g))
        while units:
            c, f = units.popleft()
            f()

    nc.compile()
    return nc


def _get_nc():
    if "nc" not in _CACHE:
        _CACHE["nc"] = _build()
    return _CACHE["nc"]


def kernel(**inputs):
    global LAST_EXEC_NS, LAST_TRACE_DIR
    from concourse.bass_utils import run_bass_kernel_spmd

    ehr = np.asarray(inputs["ehr_embeddings"], dtype=np.float32)
    mi = np.asarray(inputs["missing_indices"]).astype(np.int64)
    ei = np.asarray(inputs["exist_indices"]).astype(np.int64)
    Wq = np.asarray(inputs["Wq"], dtype=np.float32)
    Wk = np.asarray(inputs["Wk"], dtype=np.float32)
    Wv = np.asarray(inputs["Wv"], dtype=np.float32)
    bq = np.asarray(inputs["bq"], dtype=np.float32)
    bv = np.asarray(inputs["bv"], dtype=np.float32)
    cooc = np.asarray(inputs["cooc_bias"], dtype=np.float32)

    scale = 1.0 / np.sqrt(np.float32(DK))

    def fold(a):  # [E, F] -> [128, EC, F]
        return a.reshape(EC, 128, a.shape[1]).transpose(1, 0, 2)

    def wfold(a):  # [E, TT] -> [128, PP, EC, 128] (pair-col major)
        return np.ascontiguousarray(
            fold(a).reshape(128, EC, PP, 128).transpose(0, 2, 1, 3))

    missing_emb = ehr[mi]                       # [M, E]
    xt_m = np.ascontiguousarray(
        fold(missing_emb.T.astype(np.float16))
        .reshape(128, EC, 4, 512).transpose(0, 2, 1, 3))  # [128, 4, EC, 512]
    wq_all = (Wq * scale).astype(np.float16)
    wk_all = Wk.astype(np.float16)
    wv_all = Wv.astype(np.float16)
    mbt_all = cooc[:, mi, :].transpose(0, 2, 1).reshape(H * DK, M).astype(np.float16)
    bq_all = (bq * scale).astype(np.float32)

    in_maps = []
    for c in range(CORES):
        hg, ns = c // NSHARDS, c % NSHARDS
        hsl = slice(hg * TT, (hg + 1) * TT)
        eic = ei[ns * NLOC:(ns + 1) * NLOC]
        xte_f = fold(ehr[eic].T.astype(np.float16))  # [128, EC, NLOC]
        xt_e = np.ascontiguousarray(
            xte_f.reshape(128, EC, 3, 512).transpose(0, 2, 1, 3))
        ebt = np.ascontiguousarray(
            cooc[hg * HH:(hg + 1) * HH, eic, :].transpose(0, 2, 1)
            .reshape(HH * DK, NLOC).astype(np.float16))
        in_maps.append({
            "xt_m": xt_m,
            "mbt": np.ascontiguousarray(mbt_all[hsl]),
            "xt_e": xt_e, "ebt": ebt,
            "wq": wfold(wq_all[:, hsl]),
            "wk": wfold(wk_all[:, hsl]),
            "wv": np.ascontiguousarray(fold(wv_all[:, hsl])),
            "bq": np.ascontiguousarray(bq_all[hsl].reshape(PP, 128).T),
        })

    nc = _get_nc()
    kwargs = {}
    if os.environ.get("KERNEL_TRACE") == "1":
        import tempfile
        LAST_TRACE_DIR = tempfile.mkdtemp(prefix="kern_trace_")
        kwargs = {"trace": True, "tmpdir": LAST_TRACE_DIR}
        try:
            import ntff_shim
            ntff_shim.install()
        except ImportError:
            pass
    res = run_bass_kernel_spmd(nc, in_maps, list(range(CORES)), **kwargs)
    LAST_EXEC_NS = res.exec_time_ns

    # ---- host combine (exact softmax across the 4 key shards) ----
    num = np.zeros((H, M, DK), dtype=np.float64)
    den = np.zeros((H, M), dtype=np.float64)
    for c in range(CORES):
        hg = c // NSHARDS
        op = res.results[c]["out_p"].astype(np.float64)  # [HH, M, DK+1]
        num[hg * HH:(hg + 1) * HH] += op[:, :, :DK]
        den[hg * HH:(hg + 1) * HH] += op[:, :, DK]
    out = num / den[:, :, None]                          # [H, M, DK]
    out = out.transpose(1, 0, 2).reshape(M, TOTAL) + bv.astype(np.float64)
    result = ehr.copy()
    result[mi] = out.astype(np.float32)
    return result


# revision 22
# speedup vs baseline: 1.1742x; 1.0271x over previous
"""MultiHeadSectionAttentionImputer on 8 TRN2 NeuronCores (Bass/Tile).

Sharding: 2 head-groups x 4 key-shards. Core c handles heads
[6*(c//4), 6*(c//4)+6) and exist-keys [1536*(c%4), 1536*(c%4)+1536).
Each core:
  - projects its key shard to K,V (K = X_e @ Wk; V = X_e @ Wv with an
    appended ones column), its 6 heads only
  - projects the full missing set to Q for its 6 heads (Wq,bq pre-scaled
    by 1/sqrt(d_k) on host; bk dropped - it only shifts scores by a
    per-query constant, softmax-invariant and consistent across shards)
  - computes scoresT[key, query] per head with a fused 128-deep
    contraction: d' = [q-dims(64) | cooc-bias-dims(64)] so one matmul
    yields q.k/sqrt(dk) + mb.eb
  - exp() without max subtraction (scores bounded ~<60, safe in fp32)
  - attn @ [V | 1] accumulated over the 12 key chunks -> partial
    numerators (64 cols) + denominator per query
Host combines partials across the 4 key-shards of each head group
(exact softmax over all 6144 keys), adds bv, scatters into ehr.

Matmul inputs are fp16 (psum accumulates fp32); the attention weights
are bf16 (exp output needs fp32-like range; no max subtraction).
"""

import os
import sys
import numpy as np
from contextlib import ExitStack

sys.path.insert(0, "/opt/trn_rl_repo")

# problem constants (hardcoded; kernel.py must be self-contained)
H = 12          # total heads
DK = 64         # head dim
E = 768         # embed dim
TOTAL = H * DK  # 768
M = 2048        # missing sections
N = 6144        # existing sections
CORES = 8
HGROUPS = 2     # head groups (cores 0-3 -> heads 0-5, cores 4-7 -> 6-11)
NSHARDS = 4
HH = H // HGROUPS        # 6 heads per core
PP = HH // 2             # 3 head pairs per core
TT = HH * DK             # 384 projection cols per core
NLOC = N // NSHARDS      # 1536 keys per core
EC = E // 128            # 6 contraction chunks
NI = NLOC // 128         # 12 key chunks per core
MI = M // 128            # 16 query chunks

_CACHE = {}
LAST_EXEC_NS = None
LAST_TRACE_DIR = None


def _build():
    import concourse.bass as bass
    import concourse.tile as tile
    from concourse import bacc, mybir
    from collections import deque

    F32 = mybir.dt.float32
    FP16 = mybir.dt.float16
    BF16 = mybir.dt.bfloat16
    Exp = mybir.ActivationFunctionType.Exp

    nc = bacc.Bacc("TRN2", target_bir_lowering=False, debug=False)

    # ---- I/O (layouts chosen so every DMA is contiguous) ----
    xt_m = nc.dram_tensor("xt_m", [128, 4, EC, 512], FP16, kind="ExternalInput").ap()
    mbt = nc.dram_tensor("mbt", [HH * DK, M], FP16, kind="ExternalInput").ap()
    xt_e = nc.dram_tensor("xt_e", [128, 3, EC, 512], FP16, kind="ExternalInput").ap()
    ebt = nc.dram_tensor("ebt", [HH * DK, NLOC], FP16, kind="ExternalInput").ap()
    wq = nc.dram_tensor("wq", [128, PP, EC, 128], FP16, kind="ExternalInput").ap()
    wk = nc.dram_tensor("wk", [128, PP, EC, 128], FP16, kind="ExternalInput").ap()
    wv = nc.dram_tensor("wv", [128, EC, TT], FP16, kind="ExternalInput").ap()
    bq = nc.dram_tensor("bq", [128, PP], F32, kind="ExternalInput").ap()
    out_p = nc.dram_tensor("out_p", [HH, M, DK + 1], F32, kind="ExternalOutput").ap()

    with tile.TileContext(nc) as tc, ExitStack() as ctx:
        persist = ctx.enter_context(tc.tile_pool(name="persist", bufs=1))
        qpt_pool = ctx.enter_context(tc.tile_pool(name="qpt", bufs=4))
        attn_pool = ctx.enter_context(tc.tile_pool(name="attn", bufs=20))
        osb_pool = ctx.enter_context(tc.tile_pool(name="osb", bufs=12))
        proj_ps = ctx.enter_context(tc.tile_pool(name="proj_ps", bufs=2, space="PSUM"))
        sc_ps = ctx.enter_context(tc.tile_pool(name="sc_ps", bufs=2, space="PSUM"))
        av_ps = ctx.enter_context(tc.tile_pool(name="av_ps", bufs=2, space="PSUM"))

        # K'T per head [128, NLOC]: rows = k-dims | eb-dims (parity layout:
        # even head k at partitions 0:64, odd head k at 64:128 - avoids any
        # cross-partition copies; scores only need a consistent d' order)
        kpt = [persist.tile([128, NLOC], FP16, tag=f"kpt{h}", name=f"kpt{h}")
               for h in range(HH)]
        # V per key chunk [128, HH, DK+1] bf16, ones col at [., ., DK]
        vsb = [persist.tile([128, HH, DK + 1], BF16, tag=f"v{ni}", name=f"v{ni}")
               for ni in range(NI)]
        bq_sb = persist.tile([128, PP], F32, tag="bq")
        wk_big = persist.tile([128, PP, EC, 128], FP16, tag="wk")
        wq_big = persist.tile([128, PP, EC, 128], FP16, tag="wq")
        wv_big = persist.tile([128, EC, TT], FP16, tag="wv")
        xte_big = persist.tile([128, 3, EC, 512], FP16, tag="xte")
        xtm_big = persist.tile([128, 4, EC, 512], FP16, tag="xtm")
        q0_0 = qpt_pool.tile([128, M], FP16, tag="qpt", name="qpt0")
        q1_0 = qpt_pool.tile([128, M], FP16, tag="qpt", name="qpt1")

        # input DMAs: criticals (pair-0 / first key-third) on dedicated
        # queues, bulk remainder behind them
        nc.scalar.dma_start(bq_sb[:], bq)
        nc.sync.dma_start(wk_big[:, 0], wk[:, 0])
        nc.gpsimd.dma_start(q0_0[64:128, :], mbt[0:DK, :])
        nc.gpsimd.dma_start(q1_0[0:64, :], mbt[DK:2 * DK, :])
        nc.gpsimd.dma_start(wq_big[:, 0], wq[:, 0])
        nc.gpsimd.dma_start(kpt[0][64:128, :], ebt[0:DK, :])
        nc.gpsimd.dma_start(kpt[1][0:64, :], ebt[DK:2 * DK, :])
        # kt0's key-third split across sync+gpsimd so no single ~100GB/s
        # queue carries the whole 1.5MB on the critical path
        nc.sync.dma_start(xte_big[:, 0, 0:3], xt_e[:, 0, 0:3])
        nc.gpsimd.dma_start(xte_big[:, 0, 3:EC], xt_e[:, 0, 3:EC])
        nc.scalar.dma_start(xtm_big[:, 0], xt_m[:, 0])
        nc.scalar.dma_start(xtm_big[:, 1], xt_m[:, 1])
        nc.sync.dma_start(xte_big[:, 1], xt_e[:, 1])
        nc.gpsimd.dma_start(wv_big[:], wv[:])
        nc.scalar.dma_start(xtm_big[:, 2:4], xt_m[:, 2:4])
        nc.sync.dma_start(xte_big[:, 2], xt_e[:, 2])
        nc.gpsimd.dma_start(wk_big[:, 1:PP], wk[:, 1:PP])
        nc.gpsimd.dma_start(wq_big[:, 1:PP], wq[:, 1:PP])

        def emit_scores_exp_half(h, qt, ni, half, at):
            """scoresT half [128 keys, 1024 queries] + exp into attnT.
            [128,1024] scores psum tiles double-buffer (2 banks each) so
            ACT runs exp back-to-back with no psum-free wait."""
            ps = sc_ps.tile([128, 1024], F32, tag="sc", name="sc_ps_t")
            mo = half * 1024
            for mj in range(2):
                nc.tensor.matmul(
                    ps[:, mj * 512:(mj + 1) * 512],
                    lhsT=kpt[h][:, ni * 128:(ni + 1) * 128],
                    rhs=qt[:, mo + mj * 512:mo + (mj + 1) * 512],
                    start=True, stop=True)
            nc.scalar.activation(at[:, mo:mo + 1024], ps[:], Exp)

        def emit_av(h, attns, g):
            """out chunks [128 queries, DK+1], mi in [2g, 2g+2); the bf16
            attnT chunk is the stationary operand (fast weight load)."""
            for mi in range(2 * g, 2 * g + 2):
                ps = av_ps.tile([128, DK + 1], F32, tag="av", name="av_ps_t")
                for ni in range(NI):
                    nc.tensor.matmul(
                        ps[:], lhsT=attns[ni][:, mi * 128:(mi + 1) * 128],
                        rhs=vsb[ni][:, h, :],
                        start=(ni == 0), stop=(ni == NI - 1))
                ot = osb_pool.tile([128, DK + 1], F32, tag="osb", name="osb_t")
                nc.vector.tensor_copy(ot[:], ps[:])
                nc.sync.dma_start(out_p[h, mi * 128:(mi + 1) * 128, :], ot[:])

        # ---- unit-queue scheduler ----
        units = deque()
        qts = {0: q0_0, 1: q1_0}
        pieces = {0: set()}  # pair -> done piece ids (k0..k2, q0..q3)

        def emit_qt_quarter(p, mh, q0, q1):
            ps = proj_ps.tile([128, 512], F32, tag="proj", name="proj_qt")
            for ec in range(EC):
                nc.tensor.matmul(ps[:], lhsT=wq_big[:, p, ec, :],
                                 rhs=xtm_big[:, mh, ec, :],
                                 start=(ec == 0), stop=(ec == EC - 1))
            mo = mh * 512
            nc.vector.tensor_scalar_add(
                q0[0:64, mo:mo + 512], ps[0:64, :], bq_sb[0:64, p:p + 1])
            nc.vector.tensor_scalar_add(
                q1[64:128, mo:mo + 512], ps[64:128, :], bq_sb[64:128, p:p + 1])

        def qt_unit(p, mh):
            def f():
                pieces.setdefault(p, set()).add(f"q{mh}")
                q0, q1 = qts.get(2 * p), qts.get(2 * p + 1)
                if q0 is None:
                    q0 = qpt_pool.tile([128, M], FP16, tag="qpt", name=f"qpt{2*p}")
                    q1 = qpt_pool.tile([128, M], FP16, tag="qpt", name=f"qpt{2*p+1}")
                    h0, h1 = 2 * p, 2 * p + 1
                    nc.sync.dma_start(q0[64:128, :], mbt[h0 * DK:(h0 + 1) * DK, :])
                    nc.sync.dma_start(q1[0:64, :], mbt[h1 * DK:(h1 + 1) * DK, :])
                    qts[2 * p], qts[2 * p + 1] = q0, q1
                emit_qt_quarter(p, mh, q0, q1)
            return (1.35, f)

        def emit_kt_third(p, t):
            h0, h1 = 2 * p, 2 * p + 1
            lo = t * 512
            ps = proj_ps.tile([128, 512], F32, tag="proj", name="proj_kt")
            for ec in range(EC):
                nc.tensor.matmul(ps[:], lhsT=wk_big[:, p, ec, :],
                                 rhs=xte_big[:, t, ec, :],
                                 start=(ec == 0), stop=(ec == EC - 1))
            nc.vector.tensor_copy(kpt[h0][0:64, lo:lo + 512], ps[0:64, :])
            nc.vector.tensor_copy(kpt[h1][64:128, lo:lo + 512], ps[64:128, :])
            if t == 0 and p > 0:
                nc.sync.dma_start(kpt[h0][64:128, :], ebt[h0 * DK:(h0 + 1) * DK, :])
                nc.sync.dma_start(kpt[h1][0:64, :], ebt[h1 * DK:(h1 + 1) * DK, :])

        def kt_unit(p, t):
            def f():
                pieces.setdefault(p, set()).add(f"k{t}")
                emit_kt_third(p, t)
            return (1.35, f)

        def v_unit(ni):
            def f():
                ps = proj_ps.tile([128, TT], F32, tag="proj", name="proj_v")
                t, off = divmod(ni, 4)
                for ec in range(EC):
                    nc.tensor.matmul(
                        ps[:], lhsT=xte_big[:, t, ec, off * 128:(off + 1) * 128],
                        rhs=wv_big[:, ec, :], start=(ec == 0), stop=(ec == EC - 1))
                nc.vector.tensor_copy(
                    vsb[ni][:, :, 0:DK], ps[:].rearrange("p (h d) -> p h d", d=DK))
                nc.vector.memset(vsb[ni][:, :, DK], 1.0)
            return (1.0, f)

        def av_unit(h, attns, g):
            def f():
                emit_av(h, attns, g)
            return (0.8, f)

        def pump(budget):
            while units and budget > 0:
                c, f = units.popleft()
                f()
                budget -= c

        # minimal head-0 critical path up front: kt third0 + qt q0/q1
        emit_kt_third(0, 0)
        pieces[0].add("k0")
        qt_unit(0, 0)[1]()
        qt_unit(0, 1)[1]()
        units.append(kt_unit(0, 1))
        units.append(kt_unit(0, 2))
        units.append(qt_unit(0, 2))
        units.append(qt_unit(0, 3))
        for ni in range(NI):
            units.append(v_unit(ni))

        def need(p, ni, half):
            req = {f"k{ni // 4}", f"q{2 * half}", f"q{2 * half + 1}"}
            while not req <= pieces.get(p, set()):
                c, f = units.popleft()
                f()

        slot = 0
        for h in range(HH):
            p = h // 2
            if h % 2 == 1 and p + 1 <= PP - 1:
                # next pair's projections jump the queue (front) so the
                # even-head boundary never force-drains a big batch
                for mh in range(3, -1, -1):
                    units.appendleft(qt_unit(p + 1, mh))
                for t in range(2, -1, -1):
                    units.appendleft(kt_unit(p + 1, t))
            attns = [attn_pool.tile([128, M], BF16, tag="attn",
                                    name=f"attn_{h}_{ni}") for ni in range(NI)]
            for half in range(2):
                for ni in range(NI):
                    need(p, ni, half)
                    emit_scores_exp_half(h, qts[h], ni, half, attns[ni])
                    pump(2.0 if slot < 16 else 0.7)
                    slot += 1
            qts[h] = None  # release the qpt slot
            for g in range(8):
                units.append(av_unit(h, attns, g))
        while units:
            c, f = units.popleft()
            f()

    nc.compile()
    return nc


def _get_nc():
    if "nc" not in _CACHE:
        _CACHE["nc"] = _build()
    return _CACHE["nc"]


def kernel(**inputs):
    global LAST_EXEC_NS, LAST_TRACE_DIR
    from concourse.bass_utils import run_bass_kernel_spmd

    ehr = np.asarray(inputs["ehr_embeddings"], dtype=np.float32)
    mi = np.asarray(inputs["missing_indices"]).astype(np.int64)
    ei = np.asarray(inputs["exist_indices"]).astype(np.int64)
    Wq = np.asarray(inputs["Wq"], dtype=np.float32)
    Wk = np.asarray(inputs["Wk"], dtype=np.float32)
    Wv = np.asarray(inputs["Wv"], dtype=np.float32)
    bq = np.asarray(inputs["bq"], dtype=np.float32)
    bv = np.asarray(inputs["bv"], dtype=np.float32)
    cooc = np.asarray(inputs["cooc_bias"], dtype=np.float32)

    scale = 1.0 / np.sqrt(np.float32(DK))

    def fold(a):  # [E, F] -> [128, EC, F]
        return a.reshape(EC, 128, a.shape[1]).transpose(1, 0, 2)

    def wfold(a):  # [E, TT] -> [128, PP, EC, 128] (pair-col major)
        return np.ascontiguousarray(
            fold(a).reshape(128, EC, PP, 128).transpose(0, 2, 1, 3))

    missing_emb = ehr[mi]                       # [M, E]
    xt_m = np.ascontiguousarray(
        fold(missing_emb.T.astype(np.float16))
        .reshape(128, EC, 4, 512).transpose(0, 2, 1, 3))  # [128, 4, EC, 512]
    wq_all = (Wq * scale).astype(np.float16)
    wk_all = Wk.astype(np.float16)
    wv_all = Wv.astype(np.float16)
    mbt_all = cooc[:, mi, :].transpose(0, 2, 1).reshape(H * DK, M).astype(np.float16)
    bq_all = (bq * scale).astype(np.float32)

    in_maps = []
    for c in range(CORES):
        hg, ns = c // NSHARDS, c % NSHARDS
        hsl = slice(hg * TT, (hg + 1) * TT)
        eic = ei[ns * NLOC:(ns + 1) * NLOC]
        xte_f = fold(ehr[eic].T.astype(np.float16))  # [128, EC, NLOC]
        xt_e = np.ascontiguousarray(
            xte_f.reshape(128, EC, 3, 512).transpose(0, 2, 1, 3))
        ebt = np.ascontiguousarray(
            cooc[hg * HH:(hg + 1) * HH, eic, :].transpose(0, 2, 1)
            .reshape(HH * DK, NLOC).astype(np.float16))
        in_maps.append({
            "xt_m": xt_m,
            "mbt": np.ascontiguousarray(mbt_all[hsl]),
            "xt_e": xt_e, "ebt": ebt,
            "wq": wfold(wq_all[:, hsl]),
            "wk": wfold(wk_all[:, hsl]),
            "wv": np.ascontiguousarray(fold(wv_all[:, hsl])),
            "bq": np.ascontiguousarray(bq_all[hsl].reshape(PP, 128).T),
        })

    nc = _get_nc()
    kwargs = {}
    if os.environ.get("KERNEL_TRACE") == "1":
        import tempfile
        LAST_TRACE_DIR = tempfile.mkdtemp(prefix="kern_trace_")
        kwargs = {"trace": True, "tmpdir": LAST_TRACE_DIR}
        try:
            import ntff_shim
            ntff_shim.install()
        except ImportError:
            pass
    res = run_bass_kernel_spmd(nc, in_maps, list(range(CORES)), **kwargs)
    LAST_EXEC_NS = res.exec_time_ns

    # ---- host combine (exact softmax across the 4 key shards) ----
    num = np.zeros((H, M, DK), dtype=np.float64)
    den = np.zeros((H, M), dtype=np.float64)
    for c in range(CORES):
        hg = c // NSHARDS
        op = res.results[c]["out_p"].astype(np.float64)  # [HH, M, DK+1]
        num[hg * HH:(hg + 1) * HH] += op[:, :, :DK]
        den[hg * HH:(hg + 1) * HH] += op[:, :, DK]
    out = num / den[:, :, None]                          # [H, M, DK]
    out = out.transpose(1, 0, 2).reshape(M, TOTAL) + bv.astype(np.float64)
    result = ehr.copy()
    result[mi] = out.astype(np.float32)
    return result
